# revision 4
# baseline (speedup 1.0000x reference)
"""BatchHardLoss on 8 Trainium2 NeuronCores (Bass/Tile).

loss = mean_i log( pos_sum_i * neg_sum_i )
  W = clip(gamma * X @ X.T, -16, 16)   [B, B]
  pos_sum_i = sum_{j: t_j == t_i, j != i} exp(-W_ij)
  neg_sum_i = sum_{j: t_j != t_i} exp(+W_ij)

Strategy:
- Host sorts rows by class; in sorted order the same-class mask is a set of
  contiguous diagonal blocks, so each 128-row tile's same-class columns fit
  in a narrow window [w, w+CW).
- Rows are sharded across 8 cores (1024 rows each). Each core:
    * big pass over all 8192 columns: bf16 matmul -> PSUM -> one ACT
      exp(gamma * dot) with accum_out = row sums (no masking needed);
    * window pass (CW cols): matmul -> additive-mask exp sums to get
      pos_sum and the same-class part of the row sum (neg correction);
    * per-row loss = log(pos_sum * (rowsum - negcorr)) on device.
- Host averages the 8192 per-row values.  gamma*|dot| <= ~0.4 << 16 for this
  data (checked), so the clip is a no-op.
"""

import numpy as np
import ml_dtypes

B = 8192
D = 256
GAMMA = 0.001
NCORES = 8
P = 128                      # partitions / rows per tile
TILES = 8                    # row tiles per core (1024 rows/core)
ROWS_PER_CORE = P * TILES
KCH = 2                      # contraction chunks (D = 2*128)
GROUP = 2048                 # big-pass columns per PSUM group (4 banks)
NGROUPS = B // GROUP
MASKVAL = 49152.0            # exactly representable in bf16/f32
BIAS_POS = float(-(np.float32(GAMMA) * np.float32(MASKVAL)))

_program_cache = {}


def _build_program(cw):
    import concourse.bacc as bacc
    import concourse.tile as tile
    from concourse import mybir

    dt = mybir.dt
    Exp = mybir.ActivationFunctionType.Exp
    Log = mybir.ActivationFunctionType.Ln
    sub = mybir.AluOpType.subtract
    mult = mybir.AluOpType.mult

    nc = bacc.Bacc("TRN2", target_bir_lowering=False, debug=False,
                   num_devices=NCORES)

    xfull = nc.declare_dram_parameter("xfull", [P, KCH, B], dt.bfloat16, isOutput=False)
    xrows = nc.declare_dram_parameter("xrows", [P, KCH, ROWS_PER_CORE], dt.bfloat16, isOutput=False)
    xwin = nc.declare_dram_parameter("xwin", [P, TILES, KCH, cw], dt.bfloat16, isOutput=False)
    posm = nc.declare_dram_parameter("posm", [P, TILES, cw], dt.float32, isOutput=False)
    negm = nc.declare_dram_parameter("negm", [P, TILES, cw], dt.float32, isOutput=False)
    perrow = nc.declare_dram_parameter("perrow", [P, TILES], dt.float32, isOutput=True)

    with tile.TileContext(nc) as tc:
        with (
            tc.tile_pool(name="resident", bufs=1) as resident,
            tc.tile_pool(name="psum", bufs=2, space="PSUM") as psum_pool,
            tc.tile_pool(name="scratch", bufs=2) as scratch,
            tc.tile_pool(name="acc", bufs=1) as acc,
        ):
            xfull_sb = resident.tile([P, KCH, B], dt.bfloat16)
            xrows_sb = resident.tile([P, KCH, ROWS_PER_CORE], dt.bfloat16)
            xwin_sb = resident.tile([P, TILES, KCH, cw], dt.bfloat16)
            posm_sb = resident.tile([P, TILES, cw], dt.float32)
            negm_sb = resident.tile([P, TILES, cw], dt.float32)

            # split the big xfull DMA so early matmuls can start sooner
            for piece in range(4):
                w0 = piece * (B // 4)
                w1 = w0 + B // 4
                nc.sync.dma_start(out=xfull_sb[:, :, w0:w1], in_=xfull[:, :, w0:w1])
            nc.sync.dma_start(out=xrows_sb[:], in_=xrows[:])
            nc.sync.dma_start(out=xwin_sb[:], in_=xwin[:])
            nc.sync.dma_start(out=posm_sb[:], in_=posm[:])
            nc.sync.dma_start(out=negm_sb[:], in_=negm[:])

            bias_pos = acc.tile([P, 1], dt.float32)
            nc.vector.memset(bias_pos[:], BIAS_POS)
            rowparts = acc.tile([P, TILES, NGROUPS], dt.float32)
            possum = acc.tile([P, TILES], dt.float32)
            negcorr = acc.tile([P, TILES], dt.float32)

            for t in range(TILES):
                r0 = t * P
                # ---- big pass: full row sums of exp(gamma*dot) ----
                for g in range(NGROUPS):
                    ps = psum_pool.tile([P, GROUP], dt.float32, tag="big")
                    for k in range(KCH):
                        for m in range(GROUP // 512):
                            c0 = g * GROUP + m * 512
                            nc.tensor.matmul(
                                ps[:, m * 512:(m + 1) * 512],
                                lhsT=xrows_sb[:, k, r0:r0 + P],
                                rhs=xfull_sb[:, k, c0:c0 + 512],
                                start=(k == 0),
                                stop=(k == KCH - 1),
                            )
                    dump = scratch.tile([P, GROUP], dt.float32, tag="dump")
                    nc.scalar.activation(
                        dump[:], ps[:], Exp, scale=GAMMA,
                        accum_out=rowparts[:, t, g:g + 1],
                    )
                # ---- window pass: pos sums + same-class (neg) correction ----
                pw = psum_pool.tile([P, GROUP], dt.float32, tag="big")
                for k in range(KCH):
                    for m0 in range(0, cw, 512):
                        m1 = min(m0 + 512, cw)
                        nc.tensor.matmul(
                            pw[:, m0:m1],
                            lhsT=xrows_sb[:, k, r0:r0 + P],
                            rhs=xwin_sb[:, t, k, m0:m1],
                            start=(k == 0),
                            stop=(k == KCH - 1),
                        )
                pos_pre = scratch.tile([P, cw], dt.float32, tag="wpre")
                nc.vector.tensor_tensor(
                    out=pos_pre[:], in0=posm_sb[:, t, :], in1=pw[:, :cw], op=sub)
                wdump = scratch.tile([P, cw], dt.float32, tag="wdump")
                nc.scalar.activation(
                    wdump[:], pos_pre[:], Exp, scale=GAMMA, bias=bias_pos[:],
                    accum_out=possum[:, t:t + 1],
                )
                neg_pre = scratch.tile([P, cw], dt.float32, tag="wpre")
                nc.vector.tensor_tensor(
                    out=neg_pre[:], in0=pw[:, :cw], in1=negm_sb[:, t, :], op=sub)
                wdump2 = scratch.tile([P, cw], dt.float32, tag="wdump")
                nc.scalar.activation(
                    wdump2[:], neg_pre[:], Exp, scale=GAMMA,
                    accum_out=negcorr[:, t:t + 1],
                )

            # ---- combine: perrow = log(pos * (rowsum - negcorr)) ----
            rowsum = acc.tile([P, TILES], dt.float32)
            for t in range(TILES):
                nc.vector.reduce_sum(
                    rowsum[:, t:t + 1], rowparts[:, t, :], axis=mybir.AxisListType.X)
            negsum = acc.tile([P, TILES], dt.float32)
            nc.vector.tensor_tensor(out=negsum[:], in0=rowsum[:], in1=negcorr[:], op=sub)
            prod = acc.tile([P, TILES], dt.float32)
            nc.vector.tensor_tensor(out=prod[:], in0=possum[:], in1=negsum[:], op=mult)
            out_sb = acc.tile([P, TILES], dt.float32)
            nc.scalar.activation(out_sb[:], prod[:], Log)
            nc.sync.dma_start(out=perrow[:], in_=out_sb[:])

    nc.compile()
    return nc


def _numpy_fallback(x, t):
    x = x.astype(np.float32)
    total = 0.0
    for r0 in range(0, B, 1024):
        w = np.clip(x[r0:r0 + 1024] @ x.T * GAMMA, -16.0, 16.0)
        same = t[r0:r0 + 1024, None] == t[None, :]
        notself = np.ones_like(same)
        idx = np.arange(r0, r0 + 1024)
        notself[np.arange(1024), idx] = False
        pos = same & notself
        pos_sum = np.where(pos, np.exp(-w), 0.0).sum(axis=1)
        neg_sum = np.where(~same, np.exp(w), 0.0).sum(axis=1)
        total += np.log(pos_sum * neg_sum).sum(dtype=np.float64)
    return np.float32(total / B)


def kernel(inputs, targets):
    from concourse.bass_utils import run_bass_kernel_spmd

    x = np.asarray(inputs, dtype=np.float32)
    t = np.asarray(targets, dtype=np.int32)
    assert x.shape == (B, D) and t.shape == (B,)

    order = np.argsort(t, kind="stable")
    ts = t[order]
    xs = x[order]

    # the clip in the reference must be a no-op for our mask algebra
    max_norm2 = float((xs.astype(np.float64) ** 2).sum(axis=1).max())
    if GAMMA * max_norm2 > 8.0:
        return _numpy_fallback(x, t)

    # class windows per 128-row tile (sorted order)
    cls_start = np.searchsorted(ts, ts, side="left")
    cls_end = np.searchsorted(ts, ts, side="right")
    wins = []
    need = 0
    for r0 in range(0, B, P):
        w0 = int(cls_start[r0])
        w1 = int(cls_end[r0 + P - 1])
        need = max(need, w1 - w0)
        wins.append((w0, w1))
    cw = max(256, ((need + 127) // 128) * 128)
    if cw > 1024:
        return _numpy_fallback(x, t)

    xs_bf = xs.astype(ml_dtypes.bfloat16)
    XT = np.ascontiguousarray(xs_bf.T)                     # [256, 8192]
    xfull = np.ascontiguousarray(
        XT.reshape(KCH, P, B).transpose(1, 0, 2))          # [128, 2, 8192]

    in_maps = []
    for c in range(NCORES):
        lo = c * ROWS_PER_CORE
        hi = lo + ROWS_PER_CORE
        xrows = np.ascontiguousarray(
            XT[:, lo:hi].reshape(KCH, P, ROWS_PER_CORE).transpose(1, 0, 2))
        xwin_t = np.empty((P, TILES, KCH, cw), dtype=ml_dtypes.bfloat16)
        posm_t = np.empty((P, TILES, cw), dtype=np.float32)
        negm_t = np.empty((P, TILES, cw), dtype=np.float32)
        for ti in range(TILES):
            r0 = lo + ti * P
            w0, w1 = wins[r0 // P]
            w = min(w0, B - cw)
            assert w1 - w <= cw
            xwin_t[:, ti] = XT[:, w:w + cw].reshape(KCH, P, cw).transpose(1, 0, 2)
            rows_t = ts[r0:r0 + P]
            cols_t = ts[w:w + cw]
            same = rows_t[:, None] == cols_t[None, :]
            colidx = np.arange(w, w + cw)[None, :]
            rowidx = np.arange(r0, r0 + P)[:, None]
            pos = same & (colidx != rowidx)
            posm_t[:, ti] = np.where(pos, np.float32(MASKVAL), np.float32(0.0))
            negm_t[:, ti] = np.where(same, np.float32(0.0), np.float32(MASKVAL))
        in_maps.append({
            "xfull": xfull,
            "xrows": xrows,
            "xwin": xwin_t,
            "posm": posm_t,
            "negm": negm_t,
        })

    if cw not in _program_cache:
        _program_cache[cw] = _build_program(cw)
    nc = _program_cache[cw]

    res = run_bass_kernel_spmd(nc, in_maps, core_ids=list(range(NCORES)))
    vals = np.concatenate(
        [res.results[c]["perrow"].reshape(-1) for c in range(NCORES)])
    return np.float32(vals.astype(np.float64).mean())


# revision 9
# speedup vs baseline: 1.1556x; 1.1556x over previous
"""BatchHardLoss on 8 Trainium2 NeuronCores (Bass/Tile).

loss = mean_i log( pos_sum_i * neg_sum_i )
  W = clip(gamma * X @ X.T, -16, 16)   [B, B]
  pos_sum_i = sum_{j: t_j == t_i, j != i} exp(-W_ij)
  neg_sum_i = sum_{j: t_j != t_i} exp(+W_ij)

Strategy (v2, symmetric):
- Host sorts rows by class; same-class columns then fit a narrow window
  per 128-row tile (handled by a masked window pass for pos/negcorr).
- Rows sharded: core c owns the 1024 sorted rows [1024c, 1024c+1024).
- exp(W) is symmetric, so the full-matrix row sums S_i are computed by
  evaluating exp only on a 33-tile circulant band: every 128-row tile t
  computes its diagonal block plus blocks at column distance d=1..32.
  Each exp'd block feeds (a) its row accumulator via ACT accum_out and
  (b) its mirror column accumulator via a ones-matmul column sum on PE.
  The d=32 block is shared by both mirrors, so its exp is halved
  (ACT bias = -ln 2) and both sides contribute half.
- SPMD uniformity: each core gets its columns *rotated* so its own rows
  sit at local column 0; the band is then the same static slice pattern
  on every core. Host un-rotates the column accumulators, sums them
  across cores, adds row sums, and finishes log + mean (tiny).
- gamma*|dot| <= ~0.4 << 16 for this data (checked), so the clip is a
  no-op.
"""

import math

import numpy as np
import ml_dtypes

B = 8192
D = 256
GAMMA = 0.001
NCORES = 8
P = 128                      # partitions / rows per tile
TILES = 8                    # row tiles per core (1024 rows/core)
NTILES = B // P              # 64 global tiles
ROWS_PER_CORE = P * TILES
KCH = 2                      # contraction chunks (D = 2*128)
BAND = 32                    # column-tile distances 1..BAND
GROUP = 1536                 # band columns per PSUM group (3 banks)
MASKVAL = 49152.0            # exactly representable in bf16/f32
BIAS_POS = float(-(np.float32(GAMMA) * np.float32(MASKVAL)))
BIAS_HALF = -math.log(2.0)

_program_cache = {}


def _band_groups():
    """(start, width, acts) chunks of the 4096-col band, acts are
    (off, width, halved) ACT subranges inside the group."""
    groups = []
    total = BAND * P  # 4096
    pos = 0
    while pos < total:
        w = min(GROUP, total - pos)
        if pos + w == total:
            acts = []
            if w > P:
                acts.append((0, w - P, False))
            acts.append((w - P, P, True))      # d=32 sub-block: halved
        else:
            acts = [(0, w, False)]
        groups.append((pos, w, acts))
        pos += w
    return groups


def _build_program(cw):
    import concourse.bacc as bacc
    import concourse.tile as tile
    from concourse import mybir

    dt = mybir.dt
    Exp = mybir.ActivationFunctionType.Exp
    sub = mybir.AluOpType.subtract
    add = mybir.AluOpType.add

    nc = bacc.Bacc("TRN2", target_bir_lowering=False, debug=False,
                   num_devices=NCORES)

    xfull = nc.declare_dram_parameter("xfull", [P, KCH, B], dt.bfloat16, isOutput=False)
    xwin = nc.declare_dram_parameter("xwin", [P, TILES, KCH, cw], dt.bfloat16, isOutput=False)
    posm = nc.declare_dram_parameter("posm", [P, TILES, cw], dt.float32, isOutput=False)
    negm = nc.declare_dram_parameter("negm", [P, TILES, cw], dt.float32, isOutput=False)
    rows_out = nc.declare_dram_parameter("rows_out", [P, TILES], dt.float32, isOutput=True)
    possum_out = nc.declare_dram_parameter("possum_out", [P, TILES], dt.float32, isOutput=True)
    negcorr_out = nc.declare_dram_parameter("negcorr_out", [P, TILES], dt.float32, isOutput=True)
    colacc_out = nc.declare_dram_parameter("colacc_out", [P, NTILES], dt.float32, isOutput=True)

    groups = _band_groups()
    ngroups = len(groups)
    nparts = sum(len(a) for _, _, a in groups)  # rowacc slots per tile

    # col-slot bookkeeping (uniform across cores): slot jt in 1..39 is
    # touched by tiles t with max(0, jt-BAND) <= t <= min(TILES-1, jt-1)
    def slot_first(jt):
        return max(0, jt - BAND)

    def slot_last(jt):
        return min(TILES - 1, jt - 1)

    with tile.TileContext(nc) as tc:
        with (
            tc.tile_pool(name="resident", bufs=1) as resident,
            tc.tile_pool(name="psum", bufs=2, space="PSUM") as psum_pool,
            tc.tile_pool(name="cpsum", bufs=1, space="PSUM") as cpsum_pool,
            tc.tile_pool(name="scratch", bufs=3) as scratch,
            tc.tile_pool(name="acc", bufs=1) as acc,
        ):
            xfull_sb = resident.tile([P, KCH, B], dt.bfloat16)
            xwin_sb = resident.tile([P, TILES, KCH, cw], dt.bfloat16)
            posm_sb = resident.tile([P, TILES, cw], dt.float32)
            negm_sb = resident.tile([P, TILES, cw], dt.float32)

            # core's own rows (first 1024 local cols) + band columns first
            nc.sync.dma_start(out=xfull_sb[:, :, 0:2048], in_=xfull[:, :, 0:2048])
            nc.sync.dma_start(out=xfull_sb[:, :, 2048:5120], in_=xfull[:, :, 2048:5120])
            nc.sync.dma_start(out=xfull_sb[:, :, 5120:B], in_=xfull[:, :, 5120:B])
            nc.sync.dma_start(out=xwin_sb[:], in_=xwin[:])
            nc.sync.dma_start(out=posm_sb[:], in_=posm[:])
            nc.sync.dma_start(out=negm_sb[:], in_=negm[:])

            bias_pos = acc.tile([P, 1], dt.float32)
            nc.vector.memset(bias_pos[:], BIAS_POS)
            bias_half = acc.tile([P, 1], dt.float32)
            nc.vector.memset(bias_half[:], BIAS_HALF)
            ones_bf = acc.tile([P, 1], dt.bfloat16)
            nc.vector.memset(ones_bf[:], 1.0)
            zeros_bf = acc.tile([P, P], dt.bfloat16)
            nc.vector.memset(zeros_bf[:], 0.0)

            rowparts = acc.tile([P, TILES, nparts], dt.float32)
            diagrow = acc.tile([P, TILES], dt.float32)
            possum = acc.tile([P, TILES], dt.float32)
            negcorr = acc.tile([P, TILES], dt.float32)
            colacc_ps = cpsum_pool.tile([P, NTILES], dt.float32)
            # start=True clears has_written for the WHOLE bank, so it may
            # only ever happen once on this bank: zero all slots up front
            # (setting every element's has_written), then pure-accumulate.
            nc.tensor.matmul(
                colacc_ps[:, 0:NTILES],
                lhsT=zeros_bf[:, 0:P],
                rhs=zeros_bf[:, 0:NTILES],
                start=True, stop=False, skip_group_check=True,
            )

            for t in range(TILES):
                r0 = t * P
                band0 = (t + 1) * P

                # ---- diagonal block: full [128,128] row sums ----
                pd = psum_pool.tile([P, GROUP], dt.float32, tag="big")
                for k in range(KCH):
                    nc.tensor.matmul(
                        pd[:, 0:P],
                        lhsT=xfull_sb[:, k, r0:r0 + P],
                        rhs=xfull_sb[:, k, r0:r0 + P],
                        start=(k == 0), stop=(k == KCH - 1),
                    )
                ddump = scratch.tile([P, GROUP], dt.bfloat16, tag="E")
                nc.scalar.activation(
                    ddump[:, 0:P], pd[:, 0:P], Exp, scale=GAMMA,
                    accum_out=diagrow[:, t:t + 1],
                )

                # ---- circulant band d=1..32: exp once, row+col sums ----
                slot = 0
                for (g0, gw, acts) in groups:
                    ps = psum_pool.tile([P, GROUP], dt.float32, tag="big")
                    for k in range(KCH):
                        for p0 in range(0, gw, 512):
                            p1 = min(p0 + 512, gw)
                            c0 = band0 + g0 + p0
                            nc.tensor.matmul(
                                ps[:, p0:p1],
                                lhsT=xfull_sb[:, k, r0:r0 + P],
                                rhs=xfull_sb[:, k, c0:c0 + (p1 - p0)],
                                start=(k == 0), stop=(k == KCH - 1),
                            )
                    esb = scratch.tile([P, GROUP], dt.bfloat16, tag="E")
                    for (a0, aw, halved) in acts:
                        nc.scalar.activation(
                            esb[:, a0:a0 + aw], ps[:, a0:a0 + aw], Exp,
                            scale=GAMMA,
                            bias=bias_half[:] if halved else 0.0,
                            accum_out=rowparts[:, t, slot:slot + 1],
                        )
                        slot += 1
                    for csub in range(gw // P):
                        jt = t + 1 + (g0 // P) + csub
                        nc.tensor.matmul(
                            colacc_ps[:, jt:jt + 1],
                            lhsT=esb[:, csub * P:(csub + 1) * P],
                            rhs=ones_bf[:, 0:1],
                            start=False,
                            stop=(t == slot_last(jt)),
                            skip_group_check=True,
                        )

                # ---- window pass: pos sums + same-class neg correction ----
                pw = psum_pool.tile([P, GROUP], dt.float32, tag="big")
                for k in range(KCH):
                    for m0 in range(0, cw, 512):
                        m1 = min(m0 + 512, cw)
                        nc.tensor.matmul(
                            pw[:, m0:m1],
                            lhsT=xfull_sb[:, k, r0:r0 + P],
                            rhs=xwin_sb[:, t, k, m0:m1],
                            start=(k == 0), stop=(k == KCH - 1),
                        )
                pos_pre = scratch.tile([P, cw], dt.float32, tag="wpre")
                nc.vector.tensor_tensor(
                    out=pos_pre[:], in0=posm_sb[:, t, :], in1=pw[:, :cw], op=sub)
                wdump = scratch.tile([P, cw], dt.float32, tag="wdump")
                nc.scalar.activation(
                    wdump[:], pos_pre[:], Exp, scale=GAMMA, bias=bias_pos[:],
                    accum_out=possum[:, t:t + 1],
                )
                neg_pre = scratch.tile([P, cw], dt.float32, tag="wpre")
                nc.vector.tensor_tensor(
                    out=neg_pre[:], in0=pw[:, :cw], in1=negm_sb[:, t, :], op=sub)
                wdump2 = scratch.tile([P, cw], dt.float32, tag="wdump")
                nc.scalar.activation(
                    wdump2[:], neg_pre[:], Exp, scale=GAMMA,
                    accum_out=negcorr[:, t:t + 1],
                )

            # ---- wrap up: rowS = sum(rowparts) + diagrow; export ----
            rowsum = acc.tile([P, TILES], dt.float32)
            for t in range(TILES):
                nc.vector.reduce_sum(
                    rowsum[:, t:t + 1], rowparts[:, t, :], axis=mybir.AxisListType.X)
            rows_sb = acc.tile([P, TILES], dt.float32)
            nc.vector.tensor_tensor(out=rows_sb[:], in0=rowsum[:], in1=diagrow[:], op=add)
            colacc_sb = acc.tile([P, NTILES], dt.float32)
            nc.vector.tensor_copy(colacc_sb[:], colacc_ps[:])
            nc.sync.dma_start(out=rows_out[:], in_=rows_sb[:])
            nc.sync.dma_start(out=possum_out[:], in_=possum[:])
            nc.sync.dma_start(out=negcorr_out[:], in_=negcorr[:])
            nc.sync.dma_start(out=colacc_out[:], in_=colacc_sb[:])

    nc.compile()
    return nc


def _numpy_fallback(x, t):
    x = x.astype(np.float32)
    total = 0.0
    for r0 in range(0, B, 1024):
        w = np.clip(x[r0:r0 + 1024] @ x.T * GAMMA, -16.0, 16.0)
        same = t[r0:r0 + 1024, None] == t[None, :]
        notself = np.ones_like(same)
        idx = np.arange(r0, r0 + 1024)
        notself[np.arange(1024), idx] = False
        pos = same & notself
        pos_sum = np.where(pos, np.exp(-w), 0.0).sum(axis=1)
        neg_sum = np.where(~same, np.exp(w), 0.0).sum(axis=1)
        total += np.log(pos_sum * neg_sum).sum(dtype=np.float64)
    return np.float32(total / B)


def kernel(inputs, targets):
    from concourse.bass_utils import run_bass_kernel_spmd

    x = np.asarray(inputs, dtype=np.float32)
    t = np.asarray(targets, dtype=np.int32)
    assert x.shape == (B, D) and t.shape == (B,)

    order = np.argsort(t, kind="stable")
    ts = t[order]
    xs = x[order]

    # the clip in the reference must be a no-op for our mask algebra
    max_norm2 = float((xs.astype(np.float64) ** 2).sum(axis=1).max())
    if GAMMA * max_norm2 > 8.0:
        return _numpy_fallback(x, t)

    # class windows per 128-row tile (sorted order)
    cls_start = np.searchsorted(ts, ts, side="left")
    cls_end = np.searchsorted(ts, ts, side="right")
    wins = []
    need = 0
    for r0 in range(0, B, P):
        w0 = int(cls_start[r0])
        w1 = int(cls_end[r0 + P - 1])
        need = max(need, w1 - w0)
        wins.append((w0, w1))
    cw = max(256, ((need + 127) // 128) * 128)
    if cw > 1024:
        return _numpy_fallback(x, t)

    xs_bf = xs.astype(ml_dtypes.bfloat16)
    XT = np.ascontiguousarray(xs_bf.T)                     # [256, 8192]
    xfull_g = np.ascontiguousarray(
        XT.reshape(KCH, P, B).transpose(1, 0, 2))          # [128, 2, 8192]

    in_maps = []
    for c in range(NCORES):
        lo = c * ROWS_PER_CORE
        xfull_c = np.ascontiguousarray(
            np.concatenate([xfull_g[:, :, lo:], xfull_g[:, :, :lo]], axis=2))
        xwin_t = np.empty((P, TILES, KCH, cw), dtype=ml_dtypes.bfloat16)
        posm_t = np.empty((P, TILES, cw), dtype=np.float32)
        negm_t = np.empty((P, TILES, cw), dtype=np.float32)
        for ti in range(TILES):
            r0 = lo + ti * P
            w0, w1 = wins[r0 // P]
            w = min(w0, B - cw)
            assert w1 - w <= cw
            xwin_t[:, ti] = XT[:, w:w + cw].reshape(KCH, P, cw).transpose(1, 0, 2)
            rows_t = ts[r0:r0 + P]
            cols_t = ts[w:w + cw]
            same = rows_t[:, None] == cols_t[None, :]
            colidx = np.arange(w, w + cw)[None, :]
            rowidx = np.arange(r0, r0 + P)[:, None]
            pos = same & (colidx != rowidx)
            posm_t[:, ti] = np.where(pos, np.float32(MASKVAL), np.float32(0.0))
            negm_t[:, ti] = np.where(same, np.float32(0.0), np.float32(MASKVAL))
        in_maps.append({
            "xfull": xfull_c,
            "xwin": xwin_t,
            "posm": posm_t,
            "negm": negm_t,
        })

    if cw not in _program_cache:
        _program_cache[cw] = _build_program(cw)
    nc = _program_cache[cw]

    res = run_bass_kernel_spmd(nc, in_maps, core_ids=list(range(NCORES)))

    # host combine: S_i = rowS_i + colacc_i  (column sums un-rotated)
    colglob = np.zeros((P, NTILES), dtype=np.float64)
    for c in range(NCORES):
        ca = res.results[c]["colacc_out"].astype(np.float64)
        for jt in range(1, TILES + BAND):
            colglob[:, (jt + TILES * c) % NTILES] += ca[:, jt]
    S = np.empty((P, NTILES), dtype=np.float64)
    possum = np.empty((P, NTILES), dtype=np.float64)
    negcorr = np.empty((P, NTILES), dtype=np.float64)
    for c in range(NCORES):
        sl = slice(c * TILES, (c + 1) * TILES)
        S[:, sl] = res.results[c]["rows_out"].astype(np.float64)
        possum[:, sl] = res.results[c]["possum_out"].astype(np.float64)
        negcorr[:, sl] = res.results[c]["negcorr_out"].astype(np.float64)
    S += colglob
    per_row = np.log(possum * (S - negcorr))
    return np.float32(per_row.mean())


# revision 10
# speedup vs baseline: 1.3474x; 1.1660x over previous
"""BatchHardLoss on 8 Trainium2 NeuronCores (Bass/Tile).

loss = mean_i log( pos_sum_i * neg_sum_i )
  W = clip(gamma * X @ X.T, -16, 16)   [B, B]
  pos_sum_i = sum_{j: t_j == t_i, j != i} exp(-W_ij)
  neg_sum_i = sum_{j: t_j != t_i} exp(+W_ij)

Strategy (v3, symmetric + lagged column sums):
- Host sorts rows by class; same-class columns then sit in a narrow
  window per 128-row tile (pos/negcorr handled by a masked window pass).
- Rows sharded: core c owns the 1024 sorted rows [1024c, 1024c+1024).
- exp(W) is symmetric: the full-matrix row sums S_i come from a 33-tile
  circulant band per row tile (own block + distances d=1..32).  Each
  exp'd block feeds its row accumulator (ACT accum_out) and its mirror
  column accumulator (ones-matmul column sums on PE).  The d=32 block is
  halved (ACT bias -ln2) since both mirror tiles compute it.
- Column-sum matmuls for tile t are emitted during tile t+1's matmul
  stream so PE never stalls waiting for tile t's ACT outputs.
- SPMD uniformity: each core's columns are rotated so its own rows sit
  at local column 0; the band is then the same static slice pattern on
  every core.  Host un-rotates/sums column accumulators and finishes
  log + mean.
- "aligned" fast path (the expected balanced-classes case): every
  tile's same-class columns lie inside its own diagonal 128-block, so
  the window pass reads the diag part of the g0 PSUM directly (no xwin
  input, no extra matmuls).
- gamma*|dot| <= ~0.4 << 16 for this data (checked), so the clip is a
  no-op.
"""

import math

import numpy as np
import ml_dtypes

B = 8192
D = 256
GAMMA = 0.001
NCORES = 8
P = 128                      # partitions / rows per tile
TILES = 8                    # row tiles per core (1024 rows/core)
NTILES = B // P              # 64 global tiles
ROWS_PER_CORE = P * TILES
KCH = 2                      # contraction chunks (D = 2*128)
BAND = 32                    # column-tile distances 1..BAND
GROUP = 1536                 # band columns per PSUM group (3 banks)
MASKVAL = 49152.0            # exactly representable in bf16/f32
BIAS_POS = float(-(np.float32(GAMMA) * np.float32(MASKVAL)))
BIAS_HALF = -math.log(2.0)

_program_cache = {}

# band covers the tile's own block + d=1..32: 33*128 = 4224 columns,
# grouped into PSUM groups of <= GROUP columns; the final 128 columns
# (the d=32 block) get a halved exp.
def _band_groups():
    groups = []
    total = (BAND + 1) * P   # 4224
    pos = 0
    while pos < total:
        w = min(GROUP, total - pos)
        if pos + w == total:
            acts = []
            if w > P:
                acts.append((0, w - P, False))
            acts.append((w - P, P, True))       # d=32 sub-block: halved
        else:
            acts = [(0, w, False)]
        groups.append((pos, w, acts))
        pos += w
    return groups


def _build_program(cw, aligned):
    import concourse.bacc as bacc
    import concourse.tile as tile
    from concourse import mybir

    dt = mybir.dt
    Exp = mybir.ActivationFunctionType.Exp
    sub = mybir.AluOpType.subtract
    add = mybir.AluOpType.add

    nc = bacc.Bacc("TRN2", target_bir_lowering=False, debug=False,
                   num_devices=NCORES)

    xfull = nc.declare_dram_parameter("xfull", [P, KCH, B], dt.bfloat16, isOutput=False)
    if not aligned:
        xwin = nc.declare_dram_parameter("xwin", [P, TILES, KCH, cw], dt.bfloat16, isOutput=False)
    posm = nc.declare_dram_parameter("posm", [P, TILES, cw], dt.float32, isOutput=False)
    negm = nc.declare_dram_parameter("negm", [P, TILES, cw], dt.float32, isOutput=False)
    rows_out = nc.declare_dram_parameter("rows_out", [P, TILES], dt.float32, isOutput=True)
    possum_out = nc.declare_dram_parameter("possum_out", [P, TILES], dt.float32, isOutput=True)
    negcorr_out = nc.declare_dram_parameter("negcorr_out", [P, TILES], dt.float32, isOutput=True)
    colacc_out = nc.declare_dram_parameter("colacc_out", [P, NTILES], dt.float32, isOutput=True)

    groups = _band_groups()
    nparts = sum(len(a) for _, _, a in groups)

    with tile.TileContext(nc) as tc:
        with (
            tc.tile_pool(name="resident", bufs=1) as resident,
            tc.tile_pool(name="psum", bufs=2, space="PSUM") as psum_pool,
            tc.tile_pool(name="cpsum", bufs=1, space="PSUM") as cpsum_pool,
            tc.tile_pool(name="escratch", bufs=6) as escratch,
            tc.tile_pool(name="scratch", bufs=2) as scratch,
            tc.tile_pool(name="acc", bufs=1) as acc,
        ):
            xfull_sb = resident.tile([P, KCH, B], dt.bfloat16)
            posm_sb = resident.tile([P, TILES, cw], dt.float32)
            negm_sb = resident.tile([P, TILES, cw], dt.float32)

            # band columns for early tiles first
            nc.sync.dma_start(out=xfull_sb[:, :, 0:2048], in_=xfull[:, :, 0:2048])
            nc.sync.dma_start(out=xfull_sb[:, :, 2048:5120], in_=xfull[:, :, 2048:5120])
            nc.sync.dma_start(out=xfull_sb[:, :, 5120:B], in_=xfull[:, :, 5120:B])
            if not aligned:
                xwin_sb = resident.tile([P, TILES, KCH, cw], dt.bfloat16)
                nc.sync.dma_start(out=xwin_sb[:], in_=xwin[:])
            nc.sync.dma_start(out=posm_sb[:], in_=posm[:])
            nc.sync.dma_start(out=negm_sb[:], in_=negm[:])

            bias_pos = acc.tile([P, 1], dt.float32)
            nc.vector.memset(bias_pos[:], BIAS_POS)
            bias_half = acc.tile([P, 1], dt.float32)
            nc.vector.memset(bias_half[:], BIAS_HALF)
            ones_bf = acc.tile([P, 1], dt.bfloat16)
            nc.vector.memset(ones_bf[:], 1.0)
            zeros_bf = acc.tile([P, P], dt.bfloat16)
            nc.vector.memset(zeros_bf[:], 0.0)

            rowparts = acc.tile([P, TILES, nparts], dt.float32)
            possum = acc.tile([P, TILES], dt.float32)
            negcorr = acc.tile([P, TILES], dt.float32)
            colacc_ps = cpsum_pool.tile([P, NTILES], dt.float32)
            # start=True clears has_written for the WHOLE bank, so it may
            # only ever happen once on this bank: zero all slots up front
            # (setting every element's has_written), then pure-accumulate.
            nc.tensor.matmul(
                colacc_ps[:, 0:NTILES],
                lhsT=zeros_bf[:, 0:P],
                rhs=zeros_bf[:, 0:NTILES],
                start=True, stop=False, skip_group_check=True,
            )

            # tile t's colsum work, deferred into tile t+1's stream:
            # list of (esb, local sub offset, jt slot)
            pending = []

            def flush_pending(final):
                for (esb, soff, jt, last) in pending:
                    nc.tensor.matmul(
                        colacc_ps[:, jt:jt + 1],
                        lhsT=esb[:, soff:soff + P],
                        rhs=ones_bf[:, 0:1],
                        start=False,
                        stop=(last and final),
                        skip_group_check=True,
                    )
                pending.clear()

            for t in range(TILES):
                r0 = t * P
                slot = 0
                tile_pend = []
                for gi, (g0, gw, acts) in enumerate(groups):
                    ps = psum_pool.tile([P, GROUP], dt.float32, tag="big")
                    for k in range(KCH):
                        for p0 in range(0, gw, 512):
                            p1 = min(p0 + 512, gw)
                            c0 = r0 + g0 + p0
                            nc.tensor.matmul(
                                ps[:, p0:p1],
                                lhsT=xfull_sb[:, k, r0:r0 + P],
                                rhs=xfull_sb[:, k, c0:c0 + (p1 - p0)],
                                start=(k == 0), stop=(k == KCH - 1),
                            )
                    esb = escratch.tile([P, GROUP], dt.bfloat16, tag="E")
                    for (a0, aw, halved) in acts:
                        nc.scalar.activation(
                            esb[:, a0:a0 + aw], ps[:, a0:a0 + aw], Exp,
                            scale=GAMMA,
                            bias=bias_half[:] if halved else 0.0,
                            accum_out=rowparts[:, t, slot:slot + 1],
                        )
                        slot += 1
                    for csub in range(gw // P):
                        d = (g0 // P) + csub      # distance 0..32
                        if d == 0:
                            continue              # diag block: no colsum
                        jt = t + d
                        tile_pend.append((esb, csub * P, jt, d == BAND))

                    if gi == 0:
                        # window pass, reading the diag part of g0's psum
                        # (aligned) or a separate window matmul (generic)
                        if aligned:
                            wsrc = ps[:, 0:cw]
                        else:
                            pw = psum_pool.tile([P, GROUP], dt.float32, tag="big")
                            for k in range(KCH):
                                for m0 in range(0, cw, 512):
                                    m1 = min(m0 + 512, cw)
                                    nc.tensor.matmul(
                                        pw[:, m0:m1],
                                        lhsT=xfull_sb[:, k, r0:r0 + P],
                                        rhs=xwin_sb[:, t, k, m0:m1],
                                        start=(k == 0), stop=(k == KCH - 1),
                                    )
                            wsrc = pw[:, 0:cw]
                        pos_pre = scratch.tile([P, cw], dt.float32, tag="wpre")
                        nc.vector.tensor_tensor(
                            out=pos_pre[:], in0=posm_sb[:, t, :], in1=wsrc, op=sub)
                        wdump = scratch.tile([P, cw], dt.float32, tag="wdump")
                        nc.scalar.activation(
                            wdump[:], pos_pre[:], Exp, scale=GAMMA, bias=bias_pos[:],
                            accum_out=possum[:, t:t + 1],
                        )
                        neg_pre = scratch.tile([P, cw], dt.float32, tag="wpre")
                        nc.vector.tensor_tensor(
                            out=neg_pre[:], in0=wsrc, in1=negm_sb[:, t, :], op=sub)
                        wdump2 = scratch.tile([P, cw], dt.float32, tag="wdump")
                        nc.scalar.activation(
                            wdump2[:], neg_pre[:], Exp, scale=GAMMA,
                            accum_out=negcorr[:, t:t + 1],
                        )
                        # previous tile's colsums ride behind this tile's
                        # first matmul group
                        flush_pending(final=False)
                pending = tile_pend
            flush_pending(final=True)

            # ---- wrap up ----
            rowsum = acc.tile([P, TILES], dt.float32)
            for t in range(TILES):
                nc.vector.reduce_sum(
                    rowsum[:, t:t + 1], rowparts[:, t, :], axis=mybir.AxisListType.X)
            colacc_sb = acc.tile([P, NTILES], dt.float32)
            nc.vector.tensor_copy(colacc_sb[:], colacc_ps[:])
            nc.sync.dma_start(out=rows_out[:], in_=rowsum[:])
            nc.sync.dma_start(out=possum_out[:], in_=possum[:])
            nc.sync.dma_start(out=negcorr_out[:], in_=negcorr[:])
            nc.sync.dma_start(out=colacc_out[:], in_=colacc_sb[:])

    nc.compile()
    return nc


def _numpy_fallback(x, t):
    x = x.astype(np.float32)
    total = 0.0
    for r0 in range(0, B, 1024):
        w = np.clip(x[r0:r0 + 1024] @ x.T * GAMMA, -16.0, 16.0)
        same = t[r0:r0 + 1024, None] == t[None, :]
        notself = np.ones_like(same)
        idx = np.arange(r0, r0 + 1024)
        notself[np.arange(1024), idx] = False
        pos = same & notself
        pos_sum = np.where(pos, np.exp(-w), 0.0).sum(axis=1)
        neg_sum = np.where(~same, np.exp(w), 0.0).sum(axis=1)
        total += np.log(pos_sum * neg_sum).sum(dtype=np.float64)
    return np.float32(total / B)


def kernel(inputs, targets):
    from concourse.bass_utils import run_bass_kernel_spmd

    x = np.asarray(inputs, dtype=np.float32)
    t = np.asarray(targets, dtype=np.int32)
    assert x.shape == (B, D) and t.shape == (B,)

    order = np.argsort(t, kind="stable")
    ts = t[order]
    xs = x[order]

    # the clip in the reference must be a no-op for our mask algebra
    max_norm2 = float((xs.astype(np.float64) ** 2).sum(axis=1).max())
    if GAMMA * max_norm2 > 8.0:
        return _numpy_fallback(x, t)

    # class windows per 128-row tile (sorted order)
    cls_start = np.searchsorted(ts, ts, side="left")
    cls_end = np.searchsorted(ts, ts, side="right")
    wins = []
    need = 0
    aligned = True
    for r0 in range(0, B, P):
        w0 = int(cls_start[r0])
        w1 = int(cls_end[r0 + P - 1])
        need = max(need, w1 - w0)
        if w0 < r0 or w1 > r0 + P:
            aligned = False
        wins.append((w0, w1))
    if aligned:
        cw = P
    else:
        cw = max(256, ((need + 127) // 128) * 128)
        if cw > 1024:
            return _numpy_fallback(x, t)

    xs_bf = xs.astype(ml_dtypes.bfloat16)
    XT = np.ascontiguousarray(xs_bf.T)                     # [256, 8192]
    xfull_g = np.ascontiguousarray(
        XT.reshape(KCH, P, B).transpose(1, 0, 2))          # [128, 2, 8192]

    in_maps = []
    for c in range(NCORES):
        lo = c * ROWS_PER_CORE
        xfull_c = np.ascontiguousarray(
            np.concatenate([xfull_g[:, :, lo:], xfull_g[:, :, :lo]], axis=2))
        posm_t = np.empty((P, TILES, cw), dtype=np.float32)
        negm_t = np.empty((P, TILES, cw), dtype=np.float32)
        if not aligned:
            xwin_t = np.empty((P, TILES, KCH, cw), dtype=ml_dtypes.bfloat16)
        for ti in range(TILES):
            r0 = lo + ti * P
            if aligned:
                w = r0
            else:
                w0, w1 = wins[r0 // P]
                w = min(w0, B - cw)
                assert w1 - w <= cw
                xwin_t[:, ti] = XT[:, w:w + cw].reshape(KCH, P, cw).transpose(1, 0, 2)
            rows_t = ts[r0:r0 + P]
            cols_t = ts[w:w + cw]
            same = rows_t[:, None] == cols_t[None, :]
            colidx = np.arange(w, w + cw)[None, :]
            rowidx = np.arange(r0, r0 + P)[:, None]
            pos = same & (colidx != rowidx)
            posm_t[:, ti] = np.where(pos, np.float32(MASKVAL), np.float32(0.0))
            negm_t[:, ti] = np.where(same, np.float32(0.0), np.float32(MASKVAL))
        im = {"xfull": xfull_c, "posm": posm_t, "negm": negm_t}
        if not aligned:
            im["xwin"] = xwin_t
        in_maps.append(im)

    key = (cw, aligned)
    if key not in _program_cache:
        _program_cache[key] = _build_program(cw, aligned)
    nc = _program_cache[key]

    res = run_bass_kernel_spmd(nc, in_maps, core_ids=list(range(NCORES)))

    # host combine: S_i = rowS_i + colacc_i  (column sums un-rotated)
    colglob = np.zeros((P, NTILES), dtype=np.float64)
    for c in range(NCORES):
        ca = res.results[c]["colacc_out"].astype(np.float64)
        for jt in range(1, TILES + BAND):
            colglob[:, (jt + TILES * c) % NTILES] += ca[:, jt]
    S = np.empty((P, NTILES), dtype=np.float64)
    possum = np.empty((P, NTILES), dtype=np.float64)
    negcorr = np.empty((P, NTILES), dtype=np.float64)
    for c in range(NCORES):
        sl = slice(c * TILES, (c + 1) * TILES)
        S[:, sl] = res.results[c]["rows_out"].astype(np.float64)
        possum[:, sl] = res.results[c]["possum_out"].astype(np.float64)
        negcorr[:, sl] = res.results[c]["negcorr_out"].astype(np.float64)
    S += colglob
    per_row = np.log(possum * (S - negcorr))
    return np.float32(per_row.mean())


# revision 11
# speedup vs baseline: 1.4979x; 1.1116x over previous
"""BatchHardLoss on 8 Trainium2 NeuronCores (Bass/Tile).

loss = mean_i log( pos_sum_i * neg_sum_i )
  W = clip(gamma * X @ X.T, -16, 16)   [B, B]
  pos_sum_i = sum_{j: t_j == t_i, j != i} exp(-W_ij)
  neg_sum_i = sum_{j: t_j != t_i} exp(+W_ij)

Strategy (v3, symmetric + lagged column sums):
- Host sorts rows by class; same-class columns then sit in a narrow
  window per 128-row tile (pos/negcorr handled by a masked window pass).
- Rows sharded: core c owns the 1024 sorted rows [1024c, 1024c+1024).
- exp(W) is symmetric: the full-matrix row sums S_i come from a 33-tile
  circulant band per row tile (own block + distances d=1..32).  Each
  exp'd block feeds its row accumulator (ACT accum_out) and its mirror
  column accumulator (ones-matmul column sums on PE).  The d=32 block is
  halved (ACT bias -ln2) since both mirror tiles compute it.
- Column-sum matmuls for tile t are emitted during tile t+1's matmul
  stream so PE never stalls waiting for tile t's ACT outputs.
- SPMD uniformity: each core's columns are rotated so its own rows sit
  at local column 0; the band is then the same static slice pattern on
  every core.  Host un-rotates/sums column accumulators and finishes
  log + mean.
- "aligned" fast path (the expected balanced-classes case): every
  tile's same-class columns lie inside its own diagonal 128-block, so
  the window pass reads the diag part of the g0 PSUM directly (no xwin
  input, no extra matmuls).
- gamma*|dot| <= ~0.4 << 16 for this data (checked), so the clip is a
  no-op.
"""

import math

import numpy as np
import ml_dtypes

B = 8192
D = 256
GAMMA = 0.001
NCORES = 8
P = 128                      # partitions / rows per tile
TILES = 8                    # row tiles per core (1024 rows/core)
NTILES = B // P              # 64 global tiles
ROWS_PER_CORE = P * TILES
KCH = 2                      # contraction chunks (D = 2*128)
BAND = 32                    # column-tile distances 1..BAND
GROUP = 1536                 # band columns per PSUM group (3 banks)
MASKVAL = 49152.0            # exactly representable in bf16/f32
BIAS_POS = float(-(np.float32(GAMMA) * np.float32(MASKVAL)))
BIAS_HALF = -math.log(2.0)

_program_cache = {}

# band covers the tile's own block + d=1..32: 33*128 = 4224 columns,
# grouped into PSUM groups of <= GROUP columns; the final 128 columns
# (the d=32 block) get a halved exp.
def _band_groups():
    groups = []
    total = (BAND + 1) * P   # 4224
    pos = 0
    while pos < total:
        w = min(GROUP, total - pos)
        if pos + w == total:
            acts = []
            if w > P:
                acts.append((0, w - P, False))
            acts.append((w - P, P, True))       # d=32 sub-block: halved
        else:
            acts = [(0, w, False)]
        groups.append((pos, w, acts))
        pos += w
    return groups


def _build_program(cw, aligned):
    import concourse.bacc as bacc
    import concourse.tile as tile
    from concourse import mybir

    dt = mybir.dt
    Exp = mybir.ActivationFunctionType.Exp
    sub = mybir.AluOpType.subtract
    add = mybir.AluOpType.add
    mult = mybir.AluOpType.mult

    nc = bacc.Bacc("TRN2", target_bir_lowering=False, debug=False,
                   num_devices=NCORES)

    xfull = nc.declare_dram_parameter("xfull", [P, KCH, B], dt.bfloat16, isOutput=False)
    if not aligned:
        xwin = nc.declare_dram_parameter("xwin", [P, TILES, KCH, cw], dt.bfloat16, isOutput=False)
    posm = nc.declare_dram_parameter("posm", [P, TILES, cw], dt.float32, isOutput=False)
    negm = nc.declare_dram_parameter("negm", [P, TILES, cw], dt.float32, isOutput=False)
    rows_out = nc.declare_dram_parameter("rows_out", [P, TILES], dt.float32, isOutput=True)
    possum_out = nc.declare_dram_parameter("possum_out", [P, TILES], dt.float32, isOutput=True)
    negcorr_out = nc.declare_dram_parameter("negcorr_out", [P, TILES], dt.float32, isOutput=True)
    colacc_out = nc.declare_dram_parameter("colacc_out", [P, NTILES], dt.float32, isOutput=True)

    groups = _band_groups()
    nparts = sum(len(a) for _, _, a in groups)

    with tile.TileContext(nc) as tc:
        with (
            tc.tile_pool(name="resident", bufs=1) as resident,
            tc.tile_pool(name="psum", bufs=2, space="PSUM") as psum_pool,
            tc.tile_pool(name="cpsum", bufs=1, space="PSUM") as cpsum_pool,
            tc.tile_pool(name="escratch", bufs=6) as escratch,
            tc.tile_pool(name="scratch", bufs=2) as scratch,
            tc.tile_pool(name="acc", bufs=1) as acc,
        ):
            xfull_sb = resident.tile([P, KCH, B], dt.bfloat16)
            posm_sb = resident.tile([P, TILES, cw], dt.float32)
            negm_sb = resident.tile([P, TILES, cw], dt.float32)

            # band columns for early tiles first
            nc.sync.dma_start(out=xfull_sb[:, :, 0:2048], in_=xfull[:, :, 0:2048])
            nc.sync.dma_start(out=xfull_sb[:, :, 2048:5120], in_=xfull[:, :, 2048:5120])
            nc.sync.dma_start(out=xfull_sb[:, :, 5120:B], in_=xfull[:, :, 5120:B])
            if not aligned:
                xwin_sb = resident.tile([P, TILES, KCH, cw], dt.bfloat16)
                nc.sync.dma_start(out=xwin_sb[:], in_=xwin[:])
            nc.sync.dma_start(out=posm_sb[:], in_=posm[:])
            nc.sync.dma_start(out=negm_sb[:], in_=negm[:])

            bias_pos = acc.tile([P, 1], dt.float32)
            nc.vector.memset(bias_pos[:], BIAS_POS)
            bias_half = acc.tile([P, 1], dt.float32)
            nc.vector.memset(bias_half[:], BIAS_HALF)
            ones_bf = acc.tile([P, 1], dt.bfloat16)
            nc.vector.memset(ones_bf[:], 1.0)
            zeros_bf = acc.tile([P, P], dt.bfloat16)
            nc.vector.memset(zeros_bf[:], 0.0)

            rowparts = acc.tile([P, TILES, nparts], dt.float32)
            possum = acc.tile([P, TILES], dt.float32)
            negcorr = acc.tile([P, TILES], dt.float32)
            colacc_ps = cpsum_pool.tile([P, NTILES], dt.float32)
            # start=True clears has_written for the WHOLE bank, so it may
            # only ever happen once on this bank: zero all slots up front
            # (setting every element's has_written), then pure-accumulate.
            nc.tensor.matmul(
                colacc_ps[:, 0:NTILES],
                lhsT=zeros_bf[:, 0:P],
                rhs=zeros_bf[:, 0:NTILES],
                start=True, stop=False, skip_group_check=True,
            )

            # tile t's colsum work, deferred into tile t+1's stream:
            # list of (esb, local sub offset, jt slot)
            pending = []

            def flush_pending(final):
                for (esb, soff, jt, last) in pending:
                    nc.tensor.matmul(
                        colacc_ps[:, jt:jt + 1],
                        lhsT=esb[:, soff:soff + P],
                        rhs=ones_bf[:, 0:1],
                        start=False,
                        stop=(last and final),
                        skip_group_check=True,
                    )
                pending.clear()

            for t in range(TILES):
                r0 = t * P
                slot = 0
                tile_pend = []
                for gi, (g0, gw, acts) in enumerate(groups):
                    ps = psum_pool.tile([P, GROUP], dt.float32, tag="big")
                    for k in range(KCH):
                        for p0 in range(0, gw, 512):
                            p1 = min(p0 + 512, gw)
                            c0 = r0 + g0 + p0
                            nc.tensor.matmul(
                                ps[:, p0:p1],
                                lhsT=xfull_sb[:, k, r0:r0 + P],
                                rhs=xfull_sb[:, k, c0:c0 + (p1 - p0)],
                                start=(k == 0), stop=(k == KCH - 1),
                            )
                    esb = escratch.tile([P, GROUP], dt.bfloat16, tag="E")
                    for (a0, aw, halved) in acts:
                        nc.scalar.activation(
                            esb[:, a0:a0 + aw], ps[:, a0:a0 + aw], Exp,
                            scale=GAMMA,
                            bias=bias_half[:] if halved else 0.0,
                        )
                        nc.vector.reduce_sum(
                            rowparts[:, t, slot:slot + 1], esb[:, a0:a0 + aw],
                            axis=mybir.AxisListType.X)
                        slot += 1
                    for csub in range(gw // P):
                        d = (g0 // P) + csub      # distance 0..32
                        if d == 0:
                            continue              # diag block: no colsum
                        jt = t + d
                        tile_pend.append((esb, csub * P, jt, d == BAND))

                    if gi == 0:
                        # window pass: pos/neg same-class sums from the E
                        # diag block via DVE (reciprocal for exp(-W)).
                        if aligned:
                            ewin = esb[:, 0:cw]
                        else:
                            pw = psum_pool.tile([P, GROUP], dt.float32, tag="big")
                            for k in range(KCH):
                                for m0 in range(0, cw, 512):
                                    m1 = min(m0 + 512, cw)
                                    nc.tensor.matmul(
                                        pw[:, m0:m1],
                                        lhsT=xfull_sb[:, k, r0:r0 + P],
                                        rhs=xwin_sb[:, t, k, m0:m1],
                                        start=(k == 0), stop=(k == KCH - 1),
                                    )
                            ewsb = scratch.tile([P, cw], dt.bfloat16, tag="ew")
                            nc.scalar.activation(
                                ewsb[:], pw[:, 0:cw], Exp, scale=GAMMA)
                            ewin = ewsb[:]
                        nmasked = scratch.tile([P, cw], dt.float32, tag="wpre")
                        nc.vector.tensor_tensor(
                            out=nmasked[:], in0=ewin, in1=negm_sb[:, t, :], op=mult)
                        nc.vector.reduce_sum(
                            negcorr[:, t:t + 1], nmasked[:],
                            axis=mybir.AxisListType.X)
                        recip = scratch.tile([P, cw], dt.float32, tag="wrec")
                        nc.vector.reciprocal(recip[:], ewin)
                        pmasked = scratch.tile([P, cw], dt.float32, tag="wpre")
                        nc.vector.tensor_tensor(
                            out=pmasked[:], in0=recip[:], in1=posm_sb[:, t, :], op=mult)
                        nc.vector.reduce_sum(
                            possum[:, t:t + 1], pmasked[:],
                            axis=mybir.AxisListType.X)
                        # previous tile's colsums ride behind this tile's
                        # first matmul group
                        flush_pending(final=False)
                pending = tile_pend
            flush_pending(final=True)

            # ---- wrap up ----
            rowsum = acc.tile([P, TILES], dt.float32)
            for t in range(TILES):
                nc.vector.reduce_sum(
                    rowsum[:, t:t + 1], rowparts[:, t, :], axis=mybir.AxisListType.X)
            colacc_sb = acc.tile([P, NTILES], dt.float32)
            nc.vector.tensor_copy(colacc_sb[:], colacc_ps[:])
            nc.sync.dma_start(out=rows_out[:], in_=rowsum[:])
            nc.sync.dma_start(out=possum_out[:], in_=possum[:])
            nc.sync.dma_start(out=negcorr_out[:], in_=negcorr[:])
            nc.sync.dma_start(out=colacc_out[:], in_=colacc_sb[:])

    nc.compile()
    return nc


def _numpy_fallback(x, t):
    x = x.astype(np.float32)
    total = 0.0
    for r0 in range(0, B, 1024):
        w = np.clip(x[r0:r0 + 1024] @ x.T * GAMMA, -16.0, 16.0)
        same = t[r0:r0 + 1024, None] == t[None, :]
        notself = np.ones_like(same)
        idx = np.arange(r0, r0 + 1024)
        notself[np.arange(1024), idx] = False
        pos = same & notself
        pos_sum = np.where(pos, np.exp(-w), 0.0).sum(axis=1)
        neg_sum = np.where(~same, np.exp(w), 0.0).sum(axis=1)
        total += np.log(pos_sum * neg_sum).sum(dtype=np.float64)
    return np.float32(total / B)


def kernel(inputs, targets):
    from concourse.bass_utils import run_bass_kernel_spmd

    x = np.asarray(inputs, dtype=np.float32)
    t = np.asarray(targets, dtype=np.int32)
    assert x.shape == (B, D) and t.shape == (B,)

    order = np.argsort(t, kind="stable")
    ts = t[order]
    xs = x[order]

    # the clip in the reference must be a no-op for our mask algebra
    max_norm2 = float((xs.astype(np.float64) ** 2).sum(axis=1).max())
    if GAMMA * max_norm2 > 8.0:
        return _numpy_fallback(x, t)

    # class windows per 128-row tile (sorted order)
    cls_start = np.searchsorted(ts, ts, side="left")
    cls_end = np.searchsorted(ts, ts, side="right")
    wins = []
    need = 0
    aligned = True
    for r0 in range(0, B, P):
        w0 = int(cls_start[r0])
        w1 = int(cls_end[r0 + P - 1])
        need = max(need, w1 - w0)
        if w0 < r0 or w1 > r0 + P:
            aligned = False
        wins.append((w0, w1))
    if aligned:
        cw = P
    else:
        cw = max(256, ((need + 127) // 128) * 128)
        if cw > 1024:
            return _numpy_fallback(x, t)

    xs_bf = xs.astype(ml_dtypes.bfloat16)
    XT = np.ascontiguousarray(xs_bf.T)                     # [256, 8192]
    xfull_g = np.ascontiguousarray(
        XT.reshape(KCH, P, B).transpose(1, 0, 2))          # [128, 2, 8192]

    in_maps = []
    for c in range(NCORES):
        lo = c * ROWS_PER_CORE
        xfull_c = np.ascontiguousarray(
            np.concatenate([xfull_g[:, :, lo:], xfull_g[:, :, :lo]], axis=2))
        posm_t = np.empty((P, TILES, cw), dtype=np.float32)
        negm_t = np.empty((P, TILES, cw), dtype=np.float32)
        if not aligned:
            xwin_t = np.empty((P, TILES, KCH, cw), dtype=ml_dtypes.bfloat16)
        for ti in range(TILES):
            r0 = lo + ti * P
            if aligned:
                w = r0
            else:
                w0, w1 = wins[r0 // P]
                w = min(w0, B - cw)
                assert w1 - w <= cw
                xwin_t[:, ti] = XT[:, w:w + cw].reshape(KCH, P, cw).transpose(1, 0, 2)
            rows_t = ts[r0:r0 + P]
            cols_t = ts[w:w + cw]
            same = rows_t[:, None] == cols_t[None, :]
            colidx = np.arange(w, w + cw)[None, :]
            rowidx = np.arange(r0, r0 + P)[:, None]
            pos = same & (colidx != rowidx)
            posm_t[:, ti] = pos.astype(np.float32)
            negm_t[:, ti] = same.astype(np.float32)
        im = {"xfull": xfull_c, "posm": posm_t, "negm": negm_t}
        if not aligned:
            im["xwin"] = xwin_t
        in_maps.append(im)

    key = (cw, aligned)
    if key not in _program_cache:
        _program_cache[key] = _build_program(cw, aligned)
    nc = _program_cache[key]

    res = run_bass_kernel_spmd(nc, in_maps, core_ids=list(range(NCORES)))

    # host combine: S_i = rowS_i + colacc_i  (column sums un-rotated)
    colglob = np.zeros((P, NTILES), dtype=np.float64)
    for c in range(NCORES):
        ca = res.results[c]["colacc_out"].astype(np.float64)
        for jt in range(1, TILES + BAND):
            colglob[:, (jt + TILES * c) % NTILES] += ca[:, jt]
    S = np.empty((P, NTILES), dtype=np.float64)
    possum = np.empty((P, NTILES), dtype=np.float64)
    negcorr = np.empty((P, NTILES), dtype=np.float64)
    for c in range(NCORES):
        sl = slice(c * TILES, (c + 1) * TILES)
        S[:, sl] = res.results[c]["rows_out"].astype(np.float64)
        possum[:, sl] = res.results[c]["possum_out"].astype(np.float64)
        negcorr[:, sl] = res.results[c]["negcorr_out"].astype(np.float64)
    S += colglob
    per_row = np.log(possum * (S - negcorr))
    return np.float32(per_row.mean())


# revision 12
# speedup vs baseline: 1.6716x; 1.1160x over previous
"""BatchHardLoss on 8 Trainium2 NeuronCores (Bass/Tile).

loss = mean_i log( pos_sum_i * neg_sum_i )
  W = clip(gamma * X @ X.T, -16, 16)   [B, B]
  pos_sum_i = sum_{j: t_j == t_i, j != i} exp(-W_ij)
  neg_sum_i = sum_{j: t_j != t_i} exp(+W_ij)

Strategy (v3, symmetric + lagged column sums):
- Host sorts rows by class; same-class columns then sit in a narrow
  window per 128-row tile (pos/negcorr handled by a masked window pass).
- Rows sharded: core c owns the 1024 sorted rows [1024c, 1024c+1024).
- exp(W) is symmetric: the full-matrix row sums S_i come from a 33-tile
  circulant band per row tile (own block + distances d=1..32).  Each
  exp'd block feeds its row accumulator (ACT accum_out) and its mirror
  column accumulator (ones-matmul column sums on PE).  The d=32 block is
  halved (ACT bias -ln2) since both mirror tiles compute it.
- Column-sum matmuls for tile t are emitted during tile t+1's matmul
  stream so PE never stalls waiting for tile t's ACT outputs.
- SPMD uniformity: each core's columns are rotated so its own rows sit
  at local column 0; the band is then the same static slice pattern on
  every core.  Host un-rotates/sums column accumulators and finishes
  log + mean.
- "aligned" fast path (the expected balanced-classes case): every
  tile's same-class columns lie inside its own diagonal 128-block, so
  the window pass reads the diag part of the g0 PSUM directly (no xwin
  input, no extra matmuls).
- gamma*|dot| <= ~0.4 << 16 for this data (checked), so the clip is a
  no-op.
"""

import math

import numpy as np
import ml_dtypes

B = 8192
D = 256
GAMMA = 0.001
NCORES = 8
P = 128                      # partitions / rows per tile
TILES = 8                    # row tiles per core (1024 rows/core)
NTILES = B // P              # 64 global tiles
ROWS_PER_CORE = P * TILES
KCH = 2                      # contraction chunks (D = 2*128)
BAND = 32                    # column-tile distances 1..BAND
GROUP = 1536                 # band columns per PSUM group (3 banks)
MASKVAL = 49152.0            # exactly representable in bf16/f32
BIAS_POS = float(-(np.float32(GAMMA) * np.float32(MASKVAL)))
BIAS_HALF = -math.log(2.0)

_program_cache = {}

# band covers the tile's own block + d=1..32: 33*128 = 4224 columns,
# grouped into PSUM groups of <= GROUP columns; the final 128 columns
# (the d=32 block) get a halved exp.
def _band_groups():
    groups = []
    total = (BAND + 1) * P   # 4224
    pos = 0
    while pos < total:
        w = min(GROUP, total - pos)
        if pos + w == total:
            acts = []
            if w > P:
                acts.append((0, w - P, False))
            acts.append((w - P, P, True))       # d=32 sub-block: halved
        else:
            acts = [(0, w, False)]
        groups.append((pos, w, acts))
        pos += w
    return groups


def _build_program(cw, aligned):
    import concourse.bacc as bacc
    import concourse.tile as tile
    from concourse import mybir

    dt = mybir.dt
    Exp = mybir.ActivationFunctionType.Exp
    sub = mybir.AluOpType.subtract
    add = mybir.AluOpType.add
    mult = mybir.AluOpType.mult

    nc = bacc.Bacc("TRN2", target_bir_lowering=False, debug=False,
                   num_devices=NCORES)

    xfull = nc.declare_dram_parameter("xfull", [P, KCH, B], dt.bfloat16, isOutput=False)
    if not aligned:
        xwin = nc.declare_dram_parameter("xwin", [P, TILES, KCH, cw], dt.bfloat16, isOutput=False)
    posm = nc.declare_dram_parameter("posm", [P, TILES, cw], dt.float32, isOutput=False)
    negm = nc.declare_dram_parameter("negm", [P, TILES, cw], dt.float32, isOutput=False)
    rows_out = nc.declare_dram_parameter("rows_out", [P, TILES], dt.float32, isOutput=True)
    possum_out = nc.declare_dram_parameter("possum_out", [P, TILES], dt.float32, isOutput=True)
    negcorr_out = nc.declare_dram_parameter("negcorr_out", [P, TILES], dt.float32, isOutput=True)
    colacc_out = nc.declare_dram_parameter("colacc_out", [P, NTILES], dt.float32, isOutput=True)

    groups = _band_groups()
    nparts = sum(len(a) for _, _, a in groups)

    with tile.TileContext(nc) as tc:
        with (
            tc.tile_pool(name="resident", bufs=1) as resident,
            tc.tile_pool(name="psum", bufs=2, space="PSUM") as psum_pool,
            tc.tile_pool(name="cpsum", bufs=1, space="PSUM") as cpsum_pool,
            tc.tile_pool(name="escratch", bufs=6) as escratch,
            tc.tile_pool(name="scratch", bufs=2) as scratch,
            tc.tile_pool(name="acc", bufs=1) as acc,
        ):
            xfull_sb = resident.tile([P, KCH, B], dt.bfloat16)
            posm_sb = resident.tile([P, TILES, cw], dt.float32)
            negm_sb = resident.tile([P, TILES, cw], dt.float32)

            # band columns for early tiles first
            nc.sync.dma_start(out=xfull_sb[:, :, 0:1024], in_=xfull[:, :, 0:1024])
            nc.sync.dma_start(out=xfull_sb[:, :, 1024:2560], in_=xfull[:, :, 1024:2560])
            nc.sync.dma_start(out=xfull_sb[:, :, 2560:5248], in_=xfull[:, :, 2560:5248])
            nc.sync.dma_start(out=xfull_sb[:, :, 5248:B], in_=xfull[:, :, 5248:B])
            if not aligned:
                xwin_sb = resident.tile([P, TILES, KCH, cw], dt.bfloat16)
                nc.sync.dma_start(out=xwin_sb[:], in_=xwin[:])
            nc.sync.dma_start(out=posm_sb[:], in_=posm[:])
            nc.sync.dma_start(out=negm_sb[:], in_=negm[:])

            bias_pos = acc.tile([P, 1], dt.float32)
            nc.vector.memset(bias_pos[:], BIAS_POS)
            bias_half = acc.tile([P, 1], dt.float32)
            nc.vector.memset(bias_half[:], BIAS_HALF)
            ones_bf = acc.tile([P, 1], dt.bfloat16)
            nc.vector.memset(ones_bf[:], 1.0)
            zeros_bf = acc.tile([P, P], dt.bfloat16)
            nc.vector.memset(zeros_bf[:], 0.0)

            rowparts = acc.tile([P, TILES, nparts], dt.float32)
            possum = acc.tile([P, TILES], dt.float32)
            negcorr = acc.tile([P, TILES], dt.float32)
            colacc_ps = cpsum_pool.tile([P, NTILES], dt.float32)
            # start=True clears has_written for the WHOLE bank, so it may
            # only ever happen once on this bank: zero all slots up front
            # (setting every element's has_written), then pure-accumulate.
            nc.tensor.matmul(
                colacc_ps[:, 0:NTILES],
                lhsT=zeros_bf[:, 0:P],
                rhs=zeros_bf[:, 0:NTILES],
                start=True, stop=False, skip_group_check=True,
            )

            # tile t's colsum work, deferred into tile t+1's stream:
            # list of (esb, local sub offset, jt slot)
            pending = []

            def flush_pending(final):
                for (esb, soff, jt, last) in pending:
                    nc.tensor.matmul(
                        colacc_ps[:, jt:jt + 1],
                        lhsT=esb[:, soff:soff + P],
                        rhs=ones_bf[:, 0:1],
                        start=False,
                        stop=(last and final),
                        skip_group_check=True,
                    )
                pending.clear()

            for t in range(TILES):
                r0 = t * P
                slot = 0
                tile_pend = []
                for gi, (g0, gw, acts) in enumerate(groups):
                    ps = psum_pool.tile([P, GROUP], dt.float32, tag="big")
                    for k in range(KCH):
                        for p0 in range(0, gw, 512):
                            p1 = min(p0 + 512, gw)
                            c0 = r0 + g0 + p0
                            nc.tensor.matmul(
                                ps[:, p0:p1],
                                lhsT=xfull_sb[:, k, r0:r0 + P],
                                rhs=xfull_sb[:, k, c0:c0 + (p1 - p0)],
                                start=(k == 0), stop=(k == KCH - 1),
                            )
                    esb = escratch.tile([P, GROUP], dt.bfloat16, tag="E")
                    for (a0, aw, halved) in acts:
                        nc.scalar.activation(
                            esb[:, a0:a0 + aw], ps[:, a0:a0 + aw], Exp,
                            scale=GAMMA,
                            bias=bias_half[:] if halved else 0.0,
                            accum_out=rowparts[:, t, slot:slot + 1],
                        )
                        slot += 1
                    for csub in range(gw // P):
                        d = (g0 // P) + csub      # distance 0..32
                        if d == 0:
                            continue              # diag block: no colsum
                        jt = t + d
                        tile_pend.append((esb, csub * P, jt, d == BAND))

                    if gi == 0:
                        # window pass: pos/neg same-class sums from the E
                        # diag block via DVE (reciprocal for exp(-W)).
                        if aligned:
                            ewin = esb[:, 0:cw]
                        else:
                            pw = psum_pool.tile([P, GROUP], dt.float32, tag="big")
                            for k in range(KCH):
                                for m0 in range(0, cw, 512):
                                    m1 = min(m0 + 512, cw)
                                    nc.tensor.matmul(
                                        pw[:, m0:m1],
                                        lhsT=xfull_sb[:, k, r0:r0 + P],
                                        rhs=xwin_sb[:, t, k, m0:m1],
                                        start=(k == 0), stop=(k == KCH - 1),
                                    )
                            ewsb = scratch.tile([P, cw], dt.bfloat16, tag="ew")
                            nc.scalar.activation(
                                ewsb[:], pw[:, 0:cw], Exp, scale=GAMMA)
                            ewin = ewsb[:]
                        nmasked = scratch.tile([P, cw], dt.float32, tag="wpre")
                        nc.vector.tensor_tensor(
                            out=nmasked[:], in0=ewin, in1=negm_sb[:, t, :], op=mult)
                        nc.vector.reduce_sum(
                            negcorr[:, t:t + 1], nmasked[:],
                            axis=mybir.AxisListType.X)
                        recip = scratch.tile([P, cw], dt.float32, tag="wrec")
                        nc.vector.reciprocal(recip[:], ewin)
                        pmasked = scratch.tile([P, cw], dt.float32, tag="wpre")
                        nc.vector.tensor_tensor(
                            out=pmasked[:], in0=recip[:], in1=posm_sb[:, t, :], op=mult)
                        nc.vector.reduce_sum(
                            possum[:, t:t + 1], pmasked[:],
                            axis=mybir.AxisListType.X)
                        # previous tile's colsums ride behind this tile's
                        # first matmul group
                        flush_pending(final=False)
                pending = tile_pend
            flush_pending(final=True)

            # ---- wrap up ----
            rowsum = acc.tile([P, TILES], dt.float32)
            for t in range(TILES):
                nc.vector.reduce_sum(
                    rowsum[:, t:t + 1], rowparts[:, t, :], axis=mybir.AxisListType.X)
            colacc_sb = acc.tile([P, NTILES], dt.float32)
            nc.vector.tensor_copy(colacc_sb[:], colacc_ps[:])
            nc.sync.dma_start(out=rows_out[:], in_=rowsum[:])
            nc.sync.dma_start(out=possum_out[:], in_=possum[:])
            nc.sync.dma_start(out=negcorr_out[:], in_=negcorr[:])
            nc.sync.dma_start(out=colacc_out[:], in_=colacc_sb[:])

    nc.compile()
    return nc


def _numpy_fallback(x, t):
    x = x.astype(np.float32)
    total = 0.0
    for r0 in range(0, B, 1024):
        w = np.clip(x[r0:r0 + 1024] @ x.T * GAMMA, -16.0, 16.0)
        same = t[r0:r0 + 1024, None] == t[None, :]
        notself = np.ones_like(same)
        idx = np.arange(r0, r0 + 1024)
        notself[np.arange(1024), idx] = False
        pos = same & notself
        pos_sum = np.where(pos, np.exp(-w), 0.0).sum(axis=1)
        neg_sum = np.where(~same, np.exp(w), 0.0).sum(axis=1)
        total += np.log(pos_sum * neg_sum).sum(dtype=np.float64)
    return np.float32(total / B)


def kernel(inputs, targets):
    from concourse.bass_utils import run_bass_kernel_spmd

    x = np.asarray(inputs, dtype=np.float32)
    t = np.asarray(targets, dtype=np.int32)
    assert x.shape == (B, D) and t.shape == (B,)

    order = np.argsort(t, kind="stable")
    ts = t[order]
    xs = x[order]

    # the clip in the reference must be a no-op for our mask algebra
    max_norm2 = float((xs.astype(np.float64) ** 2).sum(axis=1).max())
    if GAMMA * max_norm2 > 8.0:
        return _numpy_fallback(x, t)

    # class windows per 128-row tile (sorted order)
    cls_start = np.searchsorted(ts, ts, side="left")
    cls_end = np.searchsorted(ts, ts, side="right")
    wins = []
    need = 0
    aligned = True
    for r0 in range(0, B, P):
        w0 = int(cls_start[r0])
        w1 = int(cls_end[r0 + P - 1])
        need = max(need, w1 - w0)
        if w0 < r0 or w1 > r0 + P:
            aligned = False
        wins.append((w0, w1))
    if aligned:
        cw = P
    else:
        cw = max(256, ((need + 127) // 128) * 128)
        if cw > 1024:
            return _numpy_fallback(x, t)

    xs_bf = xs.astype(ml_dtypes.bfloat16)
    XT = np.ascontiguousarray(xs_bf.T)                     # [256, 8192]
    xfull_g = np.ascontiguousarray(
        XT.reshape(KCH, P, B).transpose(1, 0, 2))          # [128, 2, 8192]

    in_maps = []
    for c in range(NCORES):
        lo = c * ROWS_PER_CORE
        xfull_c = np.ascontiguousarray(
            np.concatenate([xfull_g[:, :, lo:], xfull_g[:, :, :lo]], axis=2))
        posm_t = np.empty((P, TILES, cw), dtype=np.float32)
        negm_t = np.empty((P, TILES, cw), dtype=np.float32)
        if not aligned:
            xwin_t = np.empty((P, TILES, KCH, cw), dtype=ml_dtypes.bfloat16)
        for ti in range(TILES):
            r0 = lo + ti * P
            if aligned:
                w = r0
            else:
                w0, w1 = wins[r0 // P]
                w = min(w0, B - cw)
                assert w1 - w <= cw
                xwin_t[:, ti] = XT[:, w:w + cw].reshape(KCH, P, cw).transpose(1, 0, 2)
            rows_t = ts[r0:r0 + P]
            cols_t = ts[w:w + cw]
            same = rows_t[:, None] == cols_t[None, :]
            colidx = np.arange(w, w + cw)[None, :]
            rowidx = np.arange(r0, r0 + P)[:, None]
            pos = same & (colidx != rowidx)
            posm_t[:, ti] = pos.astype(np.float32)
            negm_t[:, ti] = same.astype(np.float32)
        im = {"xfull": xfull_c, "posm": posm_t, "negm": negm_t}
        if not aligned:
            im["xwin"] = xwin_t
        in_maps.append(im)

    key = (cw, aligned)
    if key not in _program_cache:
        _program_cache[key] = _build_program(cw, aligned)
    nc = _program_cache[key]

    res = run_bass_kernel_spmd(nc, in_maps, core_ids=list(range(NCORES)))

    # host combine: S_i = rowS_i + colacc_i  (column sums un-rotated)
    colglob = np.zeros((P, NTILES), dtype=np.float64)
    for c in range(NCORES):
        ca = res.results[c]["colacc_out"].astype(np.float64)
        for jt in range(1, TILES + BAND):
            colglob[:, (jt + TILES * c) % NTILES] += ca[:, jt]
    S = np.empty((P, NTILES), dtype=np.float64)
    possum = np.empty((P, NTILES), dtype=np.float64)
    negcorr = np.empty((P, NTILES), dtype=np.float64)
    for c in range(NCORES):
        sl = slice(c * TILES, (c + 1) * TILES)
        S[:, sl] = res.results[c]["rows_out"].astype(np.float64)
        possum[:, sl] = res.results[c]["possum_out"].astype(np.float64)
        negcorr[:, sl] = res.results[c]["negcorr_out"].astype(np.float64)
    S += colglob
    per_row = np.log(possum * (S - negcorr))
    return np.float32(per_row.mean())


# revision 13
# speedup vs baseline: 1.8042x; 1.0793x over previous
"""BatchHardLoss on 8 Trainium2 NeuronCores (Bass/Tile).

loss = mean_i log( pos_sum_i * neg_sum_i )
  W = clip(gamma * X @ X.T, -16, 16)   [B, B]
  pos_sum_i = sum_{j: t_j == t_i, j != i} exp(-W_ij)
  neg_sum_i = sum_{j: t_j != t_i} exp(+W_ij)

Strategy (v3, symmetric + lagged column sums):
- Host sorts rows by class; same-class columns then sit in a narrow
  window per 128-row tile (pos/negcorr handled by a masked window pass).
- Rows sharded: core c owns the 1024 sorted rows [1024c, 1024c+1024).
- exp(W) is symmetric: the full-matrix row sums S_i come from a 33-tile
  circulant band per row tile (own block + distances d=1..32).  Each
  exp'd block feeds its row accumulator (ACT accum_out) and its mirror
  column accumulator (ones-matmul column sums on PE).  The d=32 block is
  halved (ACT bias -ln2) since both mirror tiles compute it.
- Column-sum matmuls for tile t are emitted during tile t+1's matmul
  stream so PE never stalls waiting for tile t's ACT outputs.
- SPMD uniformity: each core's columns are rotated so its own rows sit
  at local column 0; the band is then the same static slice pattern on
  every core.  Host un-rotates/sums column accumulators and finishes
  log + mean.
- "aligned" fast path (the expected balanced-classes case): every
  tile's same-class columns lie inside its own diagonal 128-block, so
  the window pass reads the diag part of the g0 PSUM directly (no xwin
  input, no extra matmuls).
- gamma*|dot| <= ~0.4 << 16 for this data (checked), so the clip is a
  no-op.
"""

import math

import numpy as np
import ml_dtypes

B = 8192
D = 256
GAMMA = 0.001
NCORES = 8
P = 128                      # partitions / rows per tile
TILES = 8                    # row tiles per core (1024 rows/core)
NTILES = B // P              # 64 global tiles
ROWS_PER_CORE = P * TILES
KCH = 2                      # contraction chunks (D = 2*128)
BAND = 32                    # column-tile distances 1..BAND
GROUP = 1536                 # band columns per PSUM group (3 banks)
MASKVAL = 49152.0            # exactly representable in bf16/f32
BIAS_POS = float(-(np.float32(GAMMA) * np.float32(MASKVAL)))
BIAS_HALF = -math.log(2.0)

_program_cache = {}

# band covers the tile's own block + d=1..32: 33*128 = 4224 columns,
# grouped into PSUM groups of <= GROUP columns; the final 128 columns
# (the d=32 block) get a halved exp.
def _band_groups():
    groups = []
    total = (BAND + 1) * P   # 4224
    pos = 0
    while pos < total:
        w = min(GROUP, total - pos)
        groups.append((pos, w, [(0, w, False)]))
        pos += w
    return groups


def _build_program(cw, aligned):
    import concourse.bacc as bacc
    import concourse.tile as tile
    from concourse import mybir

    dt = mybir.dt
    Exp = mybir.ActivationFunctionType.Exp
    sub = mybir.AluOpType.subtract
    add = mybir.AluOpType.add
    mult = mybir.AluOpType.mult

    nc = bacc.Bacc("TRN2", target_bir_lowering=False, debug=False,
                   num_devices=NCORES)

    xfull = nc.declare_dram_parameter("xfull", [P, KCH, B], dt.bfloat16, isOutput=False)
    if not aligned:
        xwin = nc.declare_dram_parameter("xwin", [P, TILES, KCH, cw], dt.bfloat16, isOutput=False)
    posm = nc.declare_dram_parameter("posm", [P, TILES, cw], dt.float32, isOutput=False)
    negm = nc.declare_dram_parameter("negm", [P, TILES, cw], dt.float32, isOutput=False)
    rows_out = nc.declare_dram_parameter("rows_out", [P, TILES], dt.float32, isOutput=True)
    possum_out = nc.declare_dram_parameter("possum_out", [P, TILES], dt.float32, isOutput=True)
    negcorr_out = nc.declare_dram_parameter("negcorr_out", [P, TILES], dt.float32, isOutput=True)
    colacc_out = nc.declare_dram_parameter("colacc_out", [P, NTILES], dt.float32, isOutput=True)

    groups = _band_groups()
    nparts = sum(len(a) for _, _, a in groups)

    with tile.TileContext(nc) as tc:
        with (
            tc.tile_pool(name="resident", bufs=1) as resident,
            tc.tile_pool(name="psum", bufs=2, space="PSUM") as psum_pool,
            tc.tile_pool(name="cpsum", bufs=1, space="PSUM") as cpsum_pool,
            tc.tile_pool(name="escratch", bufs=6) as escratch,
            tc.tile_pool(name="scratch", bufs=2) as scratch,
            tc.tile_pool(name="acc", bufs=1) as acc,
        ):
            xfull_sb = resident.tile([P, KCH, B], dt.bfloat16)
            posm_sb = resident.tile([P, TILES, cw], dt.float32)
            negm_sb = resident.tile([P, TILES, cw], dt.float32)

            # band columns for early tiles first
            nc.sync.dma_start(out=xfull_sb[:, :, 0:1536], in_=xfull[:, :, 0:1536])
            nc.sync.dma_start(out=xfull_sb[:, :, 1536:3072], in_=xfull[:, :, 1536:3072])
            nc.sync.dma_start(out=xfull_sb[:, :, 3072:5248], in_=xfull[:, :, 3072:5248])
            nc.sync.dma_start(out=xfull_sb[:, :, 5248:B], in_=xfull[:, :, 5248:B])
            if not aligned:
                xwin_sb = resident.tile([P, TILES, KCH, cw], dt.bfloat16)
                nc.sync.dma_start(out=xwin_sb[:], in_=xwin[:])
            nc.sync.dma_start(out=posm_sb[:], in_=posm[:])
            nc.sync.dma_start(out=negm_sb[:], in_=negm[:])

            bias_pos = acc.tile([P, 1], dt.float32)
            nc.vector.memset(bias_pos[:], BIAS_POS)
            ones_bf = acc.tile([P, 1], dt.bfloat16)
            nc.vector.memset(ones_bf[:], 1.0)
            zeros_bf = acc.tile([P, P], dt.bfloat16)
            nc.vector.memset(zeros_bf[:], 0.0)

            rowparts = acc.tile([P, TILES, nparts], dt.float32)
            possum = acc.tile([P, TILES], dt.float32)
            negcorr = acc.tile([P, TILES], dt.float32)
            colacc_ps = cpsum_pool.tile([P, NTILES], dt.float32)
            # start=True clears has_written for the WHOLE bank, so it may
            # only ever happen once on this bank: zero all slots up front
            # (setting every element's has_written), then pure-accumulate.
            nc.tensor.matmul(
                colacc_ps[:, 0:NTILES],
                lhsT=zeros_bf[:, 0:P],
                rhs=zeros_bf[:, 0:NTILES],
                start=True, stop=False, skip_group_check=True,
            )

            # tile t's colsum work, deferred into tile t+1's stream:
            # list of (esb, local sub offset, jt slot)
            pending = []

            def flush_pending(final):
                for (esb, soff, jt, last) in pending:
                    nc.tensor.matmul(
                        colacc_ps[:, jt:jt + 1],
                        lhsT=esb[:, soff:soff + P],
                        rhs=ones_bf[:, 0:1],
                        start=False,
                        stop=(last and final),
                        skip_group_check=True,
                    )
                pending.clear()

            for t in range(TILES):
                r0 = t * P
                slot = 0
                tile_pend = []
                for gi, (g0, gw, acts) in enumerate(groups):
                    ps = psum_pool.tile([P, GROUP], dt.float32, tag="big")
                    for k in range(KCH):
                        for p0 in range(0, gw, 512):
                            p1 = min(p0 + 512, gw)
                            c0 = r0 + g0 + p0
                            nc.tensor.matmul(
                                ps[:, p0:p1],
                                lhsT=xfull_sb[:, k, r0:r0 + P],
                                rhs=xfull_sb[:, k, c0:c0 + (p1 - p0)],
                                start=(k == 0), stop=(k == KCH - 1),
                            )
                    esb = escratch.tile([P, GROUP], dt.bfloat16, tag="E")
                    for (a0, aw, halved) in acts:
                        nc.scalar.activation(
                            esb[:, a0:a0 + aw], ps[:, a0:a0 + aw], Exp,
                            scale=GAMMA,
                            accum_out=rowparts[:, t, slot:slot + 1],
                        )
                        slot += 1
                    for csub in range(gw // P):
                        d = (g0 // P) + csub      # distance 0..32
                        if d == 0 or d == BAND:
                            # diag: row-only.  d=32: both mirror tiles
                            # compute it row-side in full, so no colsum.
                            continue
                        jt = t + d
                        tile_pend.append((esb, csub * P, jt, d == BAND - 1))

                    if gi == 0:
                        # window pass: pos/neg same-class sums from the E
                        # diag block via DVE (reciprocal for exp(-W)).
                        if aligned:
                            ewin = esb[:, 0:cw]
                        else:
                            pw = psum_pool.tile([P, GROUP], dt.float32, tag="big")
                            for k in range(KCH):
                                for m0 in range(0, cw, 512):
                                    m1 = min(m0 + 512, cw)
                                    nc.tensor.matmul(
                                        pw[:, m0:m1],
                                        lhsT=xfull_sb[:, k, r0:r0 + P],
                                        rhs=xwin_sb[:, t, k, m0:m1],
                                        start=(k == 0), stop=(k == KCH - 1),
                                    )
                            ewsb = scratch.tile([P, cw], dt.bfloat16, tag="ew")
                            nc.scalar.activation(
                                ewsb[:], pw[:, 0:cw], Exp, scale=GAMMA)
                            ewin = ewsb[:]
                        nmasked = scratch.tile([P, cw], dt.float32, tag="wpre")
                        nc.vector.tensor_tensor(
                            out=nmasked[:], in0=ewin, in1=negm_sb[:, t, :], op=mult)
                        nc.vector.reduce_sum(
                            negcorr[:, t:t + 1], nmasked[:],
                            axis=mybir.AxisListType.X)
                        recip = scratch.tile([P, cw], dt.float32, tag="wrec")
                        nc.vector.reciprocal(recip[:], ewin)
                        pmasked = scratch.tile([P, cw], dt.float32, tag="wpre")
                        nc.vector.tensor_tensor(
                            out=pmasked[:], in0=recip[:], in1=posm_sb[:, t, :], op=mult)
                        nc.vector.reduce_sum(
                            possum[:, t:t + 1], pmasked[:],
                            axis=mybir.AxisListType.X)
                        # previous tile's colsums ride behind this tile's
                        # first matmul group
                        flush_pending(final=False)
                pending = tile_pend
            flush_pending(final=True)

            # ---- wrap up ----
            rowsum = acc.tile([P, TILES], dt.float32)
            for t in range(TILES):
                nc.vector.reduce_sum(
                    rowsum[:, t:t + 1], rowparts[:, t, :], axis=mybir.AxisListType.X)
            colacc_sb = acc.tile([P, NTILES], dt.float32)
            nc.vector.tensor_copy(colacc_sb[:], colacc_ps[:])
            nc.sync.dma_start(out=rows_out[:], in_=rowsum[:])
            nc.sync.dma_start(out=possum_out[:], in_=possum[:])
            nc.sync.dma_start(out=negcorr_out[:], in_=negcorr[:])
            nc.sync.dma_start(out=colacc_out[:], in_=colacc_sb[:])

    nc.compile()
    return nc


def _numpy_fallback(x, t):
    x = x.astype(np.float32)
    total = 0.0
    for r0 in range(0, B, 1024):
        w = np.clip(x[r0:r0 + 1024] @ x.T * GAMMA, -16.0, 16.0)
        same = t[r0:r0 + 1024, None] == t[None, :]
        notself = np.ones_like(same)
        idx = np.arange(r0, r0 + 1024)
        notself[np.arange(1024), idx] = False
        pos = same & notself
        pos_sum = np.where(pos, np.exp(-w), 0.0).sum(axis=1)
        neg_sum = np.where(~same, np.exp(w), 0.0).sum(axis=1)
        total += np.log(pos_sum * neg_sum).sum(dtype=np.float64)
    return np.float32(total / B)


def kernel(inputs, targets):
    from concourse.bass_utils import run_bass_kernel_spmd

    x = np.asarray(inputs, dtype=np.float32)
    t = np.asarray(targets, dtype=np.int32)
    assert x.shape == (B, D) and t.shape == (B,)

    order = np.argsort(t, kind="stable")
    ts = t[order]
    xs = x[order]

    # the clip in the reference must be a no-op for our mask algebra
    max_norm2 = float((xs.astype(np.float64) ** 2).sum(axis=1).max())
    if GAMMA * max_norm2 > 8.0:
        return _numpy_fallback(x, t)

    # class windows per 128-row tile (sorted order)
    cls_start = np.searchsorted(ts, ts, side="left")
    cls_end = np.searchsorted(ts, ts, side="right")
    wins = []
    need = 0
    aligned = True
    for r0 in range(0, B, P):
        w0 = int(cls_start[r0])
        w1 = int(cls_end[r0 + P - 1])
        need = max(need, w1 - w0)
        if w0 < r0 or w1 > r0 + P:
            aligned = False
        wins.append((w0, w1))
    if aligned:
        cw = P
    else:
        cw = max(256, ((need + 127) // 128) * 128)
        if cw > 1024:
            return _numpy_fallback(x, t)

    xs_bf = xs.astype(ml_dtypes.bfloat16)
    XT = np.ascontiguousarray(xs_bf.T)                     # [256, 8192]
    xfull_g = np.ascontiguousarray(
        XT.reshape(KCH, P, B).transpose(1, 0, 2))          # [128, 2, 8192]

    in_maps = []
    for c in range(NCORES):
        lo = c * ROWS_PER_CORE
        xfull_c = np.ascontiguousarray(
            np.concatenate([xfull_g[:, :, lo:], xfull_g[:, :, :lo]], axis=2))
        posm_t = np.empty((P, TILES, cw), dtype=np.float32)
        negm_t = np.empty((P, TILES, cw), dtype=np.float32)
        if not aligned:
            xwin_t = np.empty((P, TILES, KCH, cw), dtype=ml_dtypes.bfloat16)
        for ti in range(TILES):
            r0 = lo + ti * P
            if aligned:
                w = r0
            else:
                w0, w1 = wins[r0 // P]
                w = min(w0, B - cw)
                assert w1 - w <= cw
                xwin_t[:, ti] = XT[:, w:w + cw].reshape(KCH, P, cw).transpose(1, 0, 2)
            rows_t = ts[r0:r0 + P]
            cols_t = ts[w:w + cw]
            same = rows_t[:, None] == cols_t[None, :]
            colidx = np.arange(w, w + cw)[None, :]
            rowidx = np.arange(r0, r0 + P)[:, None]
            pos = same & (colidx != rowidx)
            posm_t[:, ti] = pos.astype(np.float32)
            negm_t[:, ti] = same.astype(np.float32)
        im = {"xfull": xfull_c, "posm": posm_t, "negm": negm_t}
        if not aligned:
            im["xwin"] = xwin_t
        in_maps.append(im)

    key = (cw, aligned)
    if key not in _program_cache:
        _program_cache[key] = _build_program(cw, aligned)
    nc = _program_cache[key]

    res = run_bass_kernel_spmd(nc, in_maps, core_ids=list(range(NCORES)))

    # host combine: S_i = rowS_i + colacc_i  (column sums un-rotated)
    colglob = np.zeros((P, NTILES), dtype=np.float64)
    for c in range(NCORES):
        ca = res.results[c]["colacc_out"].astype(np.float64)
        for jt in range(1, TILES + BAND - 1):
            colglob[:, (jt + TILES * c) % NTILES] += ca[:, jt]
    S = np.empty((P, NTILES), dtype=np.float64)
    possum = np.empty((P, NTILES), dtype=np.float64)
    negcorr = np.empty((P, NTILES), dtype=np.float64)
    for c in range(NCORES):
        sl = slice(c * TILES, (c + 1) * TILES)
        S[:, sl] = res.results[c]["rows_out"].astype(np.float64)
        possum[:, sl] = res.results[c]["possum_out"].astype(np.float64)
        negcorr[:, sl] = res.results[c]["negcorr_out"].astype(np.float64)
    S += colglob
    per_row = np.log(possum * (S - negcorr))
    return np.float32(per_row.mean())


# revision 14
# speedup vs baseline: 1.9019x; 1.0542x over previous
"""BatchHardLoss on 8 Trainium2 NeuronCores (Bass/Tile).

loss = mean_i log( pos_sum_i * neg_sum_i )
  W = clip(gamma * X @ X.T, -16, 16)   [B, B]
  pos_sum_i = sum_{j: t_j == t_i, j != i} exp(-W_ij)
  neg_sum_i = sum_{j: t_j != t_i} exp(+W_ij)

Strategy (v3, symmetric + lagged column sums):
- Host sorts rows by class; same-class columns then sit in a narrow
  window per 128-row tile (pos/negcorr handled by a masked window pass).
- Rows sharded: core c owns the 1024 sorted rows [1024c, 1024c+1024).
- exp(W) is symmetric: the full-matrix row sums S_i come from a 33-tile
  circulant band per row tile (own block + distances d=1..32).  Each
  exp'd block feeds its row accumulator (ACT accum_out) and its mirror
  column accumulator (ones-matmul column sums on PE).  The d=32 block is
  halved (ACT bias -ln2) since both mirror tiles compute it.
- Column-sum matmuls for tile t are emitted during tile t+1's matmul
  stream so PE never stalls waiting for tile t's ACT outputs.
- SPMD uniformity: each core's columns are rotated so its own rows sit
  at local column 0; the band is then the same static slice pattern on
  every core.  Host un-rotates/sums column accumulators and finishes
  log + mean.
- "aligned" fast path (the expected balanced-classes case): every
  tile's same-class columns lie inside its own diagonal 128-block, so
  the window pass reads the diag part of the g0 PSUM directly (no xwin
  input, no extra matmuls).
- gamma*|dot| <= ~0.4 << 16 for this data (checked), so the clip is a
  no-op.
"""

import math

import numpy as np
import ml_dtypes

B = 8192
D = 256
GAMMA = 0.001
NCORES = 8
P = 128                      # partitions / rows per tile
TILES = 8                    # row tiles per core (1024 rows/core)
NTILES = B // P              # 64 global tiles
ROWS_PER_CORE = P * TILES
KCH = 2                      # contraction chunks (D = 2*128)
BAND = 32                    # column-tile distances 1..BAND
GROUP = 1536                 # band columns per PSUM group (3 banks)
MASKVAL = 49152.0            # exactly representable in bf16/f32
BIAS_POS = float(-(np.float32(GAMMA) * np.float32(MASKVAL)))
BIAS_HALF = -math.log(2.0)

_program_cache = {}

# band covers the tile's own block + d=1..32: 33*128 = 4224 columns,
# grouped into PSUM groups of <= GROUP columns; the final 128 columns
# (the d=32 block) get a halved exp.
def _band_groups():
    groups = []
    total = (BAND + 1) * P   # 4224
    pos = 0
    while pos < total:
        w = min(GROUP, total - pos)
        groups.append((pos, w, [(0, w, False)]))
        pos += w
    return groups


def _build_program(cw, aligned):
    import concourse.bacc as bacc
    import concourse.tile as tile
    from concourse import mybir

    dt = mybir.dt
    Exp = mybir.ActivationFunctionType.Exp
    sub = mybir.AluOpType.subtract
    add = mybir.AluOpType.add
    mult = mybir.AluOpType.mult
    DR = mybir.MatmulPerfMode.DoubleRow

    nc = bacc.Bacc("TRN2", target_bir_lowering=False, debug=False,
                   num_devices=NCORES)

    xfull = nc.declare_dram_parameter("xfull", [P, KCH, B], dt.float8e4, isOutput=False)
    if not aligned:
        xwin = nc.declare_dram_parameter("xwin", [P, TILES, KCH, cw], dt.float8e4, isOutput=False)
    posm = nc.declare_dram_parameter("posm", [P, TILES, cw], dt.float32, isOutput=False)
    negm = nc.declare_dram_parameter("negm", [P, TILES, cw], dt.float32, isOutput=False)
    rows_out = nc.declare_dram_parameter("rows_out", [P, TILES], dt.float32, isOutput=True)
    possum_out = nc.declare_dram_parameter("possum_out", [P, TILES], dt.float32, isOutput=True)
    negcorr_out = nc.declare_dram_parameter("negcorr_out", [P, TILES], dt.float32, isOutput=True)
    colacc_out = nc.declare_dram_parameter("colacc_out", [P, NTILES], dt.float32, isOutput=True)

    groups = _band_groups()
    nparts = sum(len(a) for _, _, a in groups)

    with tile.TileContext(nc) as tc:
        with (
            tc.tile_pool(name="resident", bufs=1) as resident,
            tc.tile_pool(name="psum", bufs=2, space="PSUM") as psum_pool,
            tc.tile_pool(name="cpsum", bufs=1, space="PSUM") as cpsum_pool,
            tc.tile_pool(name="escratch", bufs=6) as escratch,
            tc.tile_pool(name="scratch", bufs=2) as scratch,
            tc.tile_pool(name="acc", bufs=1) as acc,
        ):
            xfull_sb = resident.tile([P, KCH, B], dt.float8e4)
            posm_sb = resident.tile([P, TILES, cw], dt.float32)
            negm_sb = resident.tile([P, TILES, cw], dt.float32)

            # band columns for early tiles first
            nc.sync.dma_start(out=xfull_sb[:, :, 0:1536], in_=xfull[:, :, 0:1536])
            nc.sync.dma_start(out=xfull_sb[:, :, 1536:3072], in_=xfull[:, :, 1536:3072])
            nc.sync.dma_start(out=xfull_sb[:, :, 3072:5248], in_=xfull[:, :, 3072:5248])
            nc.sync.dma_start(out=xfull_sb[:, :, 5248:B], in_=xfull[:, :, 5248:B])
            if not aligned:
                xwin_sb = resident.tile([P, TILES, KCH, cw], dt.float8e4)
                nc.sync.dma_start(out=xwin_sb[:], in_=xwin[:])
            nc.sync.dma_start(out=posm_sb[:], in_=posm[:])
            nc.sync.dma_start(out=negm_sb[:], in_=negm[:])

            bias_pos = acc.tile([P, 1], dt.float32)
            nc.vector.memset(bias_pos[:], BIAS_POS)
            ones_bf = acc.tile([P, 1], dt.bfloat16)
            nc.vector.memset(ones_bf[:], 1.0)
            zeros_bf = acc.tile([P, P], dt.bfloat16)
            nc.vector.memset(zeros_bf[:], 0.0)

            rowparts = acc.tile([P, TILES, nparts], dt.float32)
            possum = acc.tile([P, TILES], dt.float32)
            negcorr = acc.tile([P, TILES], dt.float32)
            colacc_ps = cpsum_pool.tile([P, NTILES], dt.float32)
            # start=True clears has_written for the WHOLE bank, so it may
            # only ever happen once on this bank: zero all slots up front
            # (setting every element's has_written), then pure-accumulate.
            nc.tensor.matmul(
                colacc_ps[:, 0:NTILES],
                lhsT=zeros_bf[:, 0:P],
                rhs=zeros_bf[:, 0:NTILES],
                start=True, stop=False, skip_group_check=True,
            )

            # tile t's colsum work, deferred into tile t+1's stream:
            # list of (esb, local sub offset, jt slot)
            pending = []

            def flush_pending(final):
                for (esb, soff, jt, last) in pending:
                    nc.tensor.matmul(
                        colacc_ps[:, jt:jt + 1],
                        lhsT=esb[:, soff:soff + P],
                        rhs=ones_bf[:, 0:1],
                        start=False,
                        stop=(last and final),
                        skip_group_check=True,
                    )
                pending.clear()

            for t in range(TILES):
                r0 = t * P
                slot = 0
                tile_pend = []
                for gi, (g0, gw, acts) in enumerate(groups):
                    ps = psum_pool.tile([P, GROUP], dt.float32, tag="big")
                    for p0 in range(0, gw, 512):
                        p1 = min(p0 + 512, gw)
                        c0 = r0 + g0 + p0
                        nc.tensor.matmul(
                            ps[:, p0:p1],
                            lhsT=xfull_sb[:, :, r0:r0 + P],
                            rhs=xfull_sb[:, :, c0:c0 + (p1 - p0)],
                            start=True, stop=True, perf_mode=DR,
                        )
                    esb = escratch.tile([P, GROUP], dt.bfloat16, tag="E")
                    for (a0, aw, halved) in acts:
                        nc.scalar.activation(
                            esb[:, a0:a0 + aw], ps[:, a0:a0 + aw], Exp,
                            scale=GAMMA,
                            accum_out=rowparts[:, t, slot:slot + 1],
                        )
                        slot += 1
                    for csub in range(gw // P):
                        d = (g0 // P) + csub      # distance 0..32
                        if d == 0 or d == BAND:
                            # diag: row-only.  d=32: both mirror tiles
                            # compute it row-side in full, so no colsum.
                            continue
                        jt = t + d
                        tile_pend.append((esb, csub * P, jt, d == BAND - 1))

                    if gi == 0:
                        # window pass: pos/neg same-class sums from the E
                        # diag block via DVE (reciprocal for exp(-W)).
                        if aligned:
                            ewin = esb[:, 0:cw]
                        else:
                            pw = psum_pool.tile([P, GROUP], dt.float32, tag="big")
                            for m0 in range(0, cw, 512):
                                m1 = min(m0 + 512, cw)
                                nc.tensor.matmul(
                                    pw[:, m0:m1],
                                    lhsT=xfull_sb[:, :, r0:r0 + P],
                                    rhs=xwin_sb[:, t, :, m0:m1],
                                    start=True, stop=True, perf_mode=DR,
                                )
                            ewsb = scratch.tile([P, cw], dt.bfloat16, tag="ew")
                            nc.scalar.activation(
                                ewsb[:], pw[:, 0:cw], Exp, scale=GAMMA)
                            ewin = ewsb[:]
                        nmasked = scratch.tile([P, cw], dt.float32, tag="wpre")
                        nc.vector.tensor_tensor(
                            out=nmasked[:], in0=ewin, in1=negm_sb[:, t, :], op=mult)
                        nc.vector.reduce_sum(
                            negcorr[:, t:t + 1], nmasked[:],
                            axis=mybir.AxisListType.X)
                        recip = scratch.tile([P, cw], dt.float32, tag="wrec")
                        nc.vector.reciprocal(recip[:], ewin)
                        pmasked = scratch.tile([P, cw], dt.float32, tag="wpre")
                        nc.vector.tensor_tensor(
                            out=pmasked[:], in0=recip[:], in1=posm_sb[:, t, :], op=mult)
                        nc.vector.reduce_sum(
                            possum[:, t:t + 1], pmasked[:],
                            axis=mybir.AxisListType.X)
                        # previous tile's colsums ride behind this tile's
                        # first matmul group
                        flush_pending(final=False)
                pending = tile_pend
            flush_pending(final=True)

            # ---- wrap up ----
            rowsum = acc.tile([P, TILES], dt.float32)
            for t in range(TILES):
                nc.vector.reduce_sum(
                    rowsum[:, t:t + 1], rowparts[:, t, :], axis=mybir.AxisListType.X)
            colacc_sb = acc.tile([P, NTILES], dt.float32)
            nc.vector.tensor_copy(colacc_sb[:], colacc_ps[:])
            nc.sync.dma_start(out=rows_out[:], in_=rowsum[:])
            nc.sync.dma_start(out=possum_out[:], in_=possum[:])
            nc.sync.dma_start(out=negcorr_out[:], in_=negcorr[:])
            nc.sync.dma_start(out=colacc_out[:], in_=colacc_sb[:])

    nc.compile()
    return nc


def _numpy_fallback(x, t):
    x = x.astype(np.float32)
    total = 0.0
    for r0 in range(0, B, 1024):
        w = np.clip(x[r0:r0 + 1024] @ x.T * GAMMA, -16.0, 16.0)
        same = t[r0:r0 + 1024, None] == t[None, :]
        notself = np.ones_like(same)
        idx = np.arange(r0, r0 + 1024)
        notself[np.arange(1024), idx] = False
        pos = same & notself
        pos_sum = np.where(pos, np.exp(-w), 0.0).sum(axis=1)
        neg_sum = np.where(~same, np.exp(w), 0.0).sum(axis=1)
        total += np.log(pos_sum * neg_sum).sum(dtype=np.float64)
    return np.float32(total / B)


def kernel(inputs, targets):
    from concourse.bass_utils import run_bass_kernel_spmd

    x = np.asarray(inputs, dtype=np.float32)
    t = np.asarray(targets, dtype=np.int32)
    assert x.shape == (B, D) and t.shape == (B,)

    order = np.argsort(t, kind="stable")
    ts = t[order]
    xs = x[order]

    # the clip in the reference must be a no-op for our mask algebra
    max_norm2 = float((xs.astype(np.float64) ** 2).sum(axis=1).max())
    if GAMMA * max_norm2 > 8.0:
        return _numpy_fallback(x, t)

    # class windows per 128-row tile (sorted order)
    cls_start = np.searchsorted(ts, ts, side="left")
    cls_end = np.searchsorted(ts, ts, side="right")
    wins = []
    need = 0
    aligned = True
    for r0 in range(0, B, P):
        w0 = int(cls_start[r0])
        w1 = int(cls_end[r0 + P - 1])
        need = max(need, w1 - w0)
        if w0 < r0 or w1 > r0 + P:
            aligned = False
        wins.append((w0, w1))
    if aligned:
        cw = P
    else:
        cw = max(256, ((need + 127) // 128) * 128)
        if cw > 1024:
            return _numpy_fallback(x, t)

    xs_q = xs.astype(ml_dtypes.float8_e4m3)
    XT = np.ascontiguousarray(xs_q.T)                      # [256, 8192]
    xfull_g = np.ascontiguousarray(
        XT.reshape(KCH, P, B).transpose(1, 0, 2))          # [128, 2, 8192]

    in_maps = []
    for c in range(NCORES):
        lo = c * ROWS_PER_CORE
        xfull_c = np.ascontiguousarray(
            np.concatenate([xfull_g[:, :, lo:], xfull_g[:, :, :lo]], axis=2))
        posm_t = np.empty((P, TILES, cw), dtype=np.float32)
        negm_t = np.empty((P, TILES, cw), dtype=np.float32)
        if not aligned:
            xwin_t = np.empty((P, TILES, KCH, cw), dtype=ml_dtypes.float8_e4m3)
        for ti in range(TILES):
            r0 = lo + ti * P
            if aligned:
                w = r0
            else:
                w0, w1 = wins[r0 // P]
                w = min(w0, B - cw)
                assert w1 - w <= cw
                xwin_t[:, ti] = XT[:, w:w + cw].reshape(KCH, P, cw).transpose(1, 0, 2)
            rows_t = ts[r0:r0 + P]
            cols_t = ts[w:w + cw]
            same = rows_t[:, None] == cols_t[None, :]
            colidx = np.arange(w, w + cw)[None, :]
            rowidx = np.arange(r0, r0 + P)[:, None]
            pos = same & (colidx != rowidx)
            posm_t[:, ti] = pos.astype(np.float32)
            negm_t[:, ti] = same.astype(np.float32)
        im = {"xfull": xfull_c, "posm": posm_t, "negm": negm_t}
        if not aligned:
            im["xwin"] = xwin_t
        in_maps.append(im)

    key = (cw, aligned)
    if key not in _program_cache:
        _program_cache[key] = _build_program(cw, aligned)
    nc = _program_cache[key]

    res = run_bass_kernel_spmd(nc, in_maps, core_ids=list(range(NCORES)))

    # host combine: S_i = rowS_i + colacc_i  (column sums un-rotated)
    colglob = np.zeros((P, NTILES), dtype=np.float64)
    for c in range(NCORES):
        ca = res.results[c]["colacc_out"].astype(np.float64)
        for jt in range(1, TILES + BAND - 1):
            colglob[:, (jt + TILES * c) % NTILES] += ca[:, jt]
    S = np.empty((P, NTILES), dtype=np.float64)
    possum = np.empty((P, NTILES), dtype=np.float64)
    negcorr = np.empty((P, NTILES), dtype=np.float64)
    for c in range(NCORES):
        sl = slice(c * TILES, (c + 1) * TILES)
        S[:, sl] = res.results[c]["rows_out"].astype(np.float64)
        possum[:, sl] = res.results[c]["possum_out"].astype(np.float64)
        negcorr[:, sl] = res.results[c]["negcorr_out"].astype(np.float64)
    S += colglob
    per_row = np.log(possum * (S - negcorr))
    return np.float32(per_row.mean())
